# revision 27
# baseline (speedup 1.0000x reference)
"""Trainium2 Bass kernel for MatrixOdeGradientDescentModel.

Reference computation (B=4096, DZ=512, H=2048, DY=10, n_steps=64):
    z = x; repeat n_steps: z += dt * z @ A.T          (dt = 1/n_steps)
    y = relu(z @ W1.T + b1) @ W2.T + b2

Algebraic rewrite: the Euler loop is linear, so
    z_final = x @ (M^T)^n  with  M = I + dt*A.
We compute W := M^T = I + dt*A^T by repeated squaring on the *deviation*
D_k := W^(2^k) - I (avoids precision loss from the identity's magnitude):
    D_{k+1} = 2*D_k + D_k @ D_k
maintaining the pair (D_k, T_k=D_k^T) so no on-device transposes are needed:
    D@D = matmul(lhsT=T, rhs=D),   (D@D)^T = matmul(lhsT=D, rhs=T)
then zT = xT + D_chain applied to xT per set bit of n (binary exponentiation).

Sharding: data-parallel over batch. Each of the 8 cores gets 512 rows of x;
A/W1/W2 replicated; no cross-core communication.

Matmuls run in float32r (TF32-like, 4x faster than fp32 on the PE) with fp32
PSUM accumulation; the error-compensated deviation chain keeps the end-to-end
relative error at the ~1e-4 level.
"""

import os

import numpy as np

import concourse.bacc as bacc
import concourse.mybir as mybir
import concourse.tile as tile
from concourse.bass_utils import run_bass_kernel_spmd

P = 128
B, DZ, H, DY = 4096, 512, 2048, 10
NCORES = 8
BC = B // NCORES          # 512 rows per core
DT = DZ // P              # 4 k-tiles over DZ
HT = H // P               # 16 m-tiles over H

f32 = mybir.dt.float32
f32r = mybir.dt.float32r

_BUILD_CACHE = {}


def _emit_mm_set(nc, psum_pool, lhsT_tile, rhs_tile, evict, n_mt=DT):
    """One [512,512]-ish matmul set: for each output row-block mt, accumulate
    over DT k-tiles into PSUM and call evict(mt, psum_ap)."""
    for mt in range(n_mt):
        ps = psum_pool.tile([P, BC], f32, tag="ps")
        for kt in range(DT):
            nc.tensor.matmul(
                ps[:],
                lhsT_tile[:, kt, mt * P:(mt + 1) * P],
                rhs_tile[:, kt, :],
                start=(kt == 0),
                stop=(kt == DT - 1),
            )
        evict(mt, ps)


def _build(n_steps: int):
    """Build + compile the Bass module for a given n_steps."""
    n = int(n_steps)
    assert n >= 0
    nc = bacc.Bacc("TRN2", target_bir_lowering=False, debug=False,
                   enable_asserts=False, num_devices=NCORES)

    # f32r-declared DRAM inputs carry raw fp32 bytes; the PE rounds internally
    # (verified bit-identical to an explicit cast) so plain HWDGE DMA works.
    xt_d = nc.dram_tensor("xt", [P, DT * BC], f32, kind="ExternalInput")
    xtr_d = nc.dram_tensor("xtr", [P, DT * BC], f32r, kind="ExternalInput")
    t0_d = nc.dram_tensor("t0", [P, DT * DZ], f32r, kind="ExternalInput")
    w1t_d = nc.dram_tensor("w1t", [P, DT * H], f32r, kind="ExternalInput")
    b1t_d = nc.dram_tensor("b1t", [P, HT], f32, kind="ExternalInput")
    w2t_d = nc.dram_tensor("w2t", [P, HT * DY], f32r, kind="ExternalInput")
    b2t_d = nc.dram_tensor("b2t", [DY, 1], f32, kind="ExternalInput")
    ident_d = nc.dram_tensor("ident", [P, P], f32, kind="ExternalInput")
    identr_d = nc.dram_tensor("identr", [P, P], f32r, kind="ExternalInput")
    y_d = nc.dram_tensor("y", [BC, DY], f32, kind="ExternalOutput")

    mult = mybir.AluOpType.mult
    add = mybir.AluOpType.add

    with tile.TileContext(nc) as tc:
        with (
            tc.tile_pool(name="const", bufs=1) as const_pool,
            tc.tile_pool(name="weights", bufs=1) as w_pool,
            tc.tile_pool(name="chain", bufs=2) as chain_pool,
            tc.tile_pool(name="accp", bufs=2) as acc_pool,
            tc.tile_pool(name="acts", bufs=1) as act_pool,
            tc.tile_pool(name="out", bufs=2) as out_pool,
            tc.tile_pool(name="psum", bufs=7, space="PSUM") as psum_pool,
            tc.tile_pool(name="psum_y", bufs=1, space="PSUM") as psum_y_pool,
        ):
            # ---- loads (all fast HWDGE; chain inputs first) ----------------
            def load(dram, shape, tag, dtype=f32r, chunks=1):
                r = w_pool.tile(shape, dtype, tag=tag)
                src = dram.ap().rearrange("p (t b) -> p t b", t=shape[1])
                for ch in range(chunks):
                    lo = shape[1] * ch // chunks
                    hi = shape[1] * (ch + 1) // chunks
                    nc.sync.dma_start(r[:, lo:hi, :], src[:, lo:hi, :])
                return r

            # All loads go through one trigger queue (Sync) in priority order:
            # the DMA rings are FIFO, so t0 — which gates the squaring
            # chain — must be enqueued before the bulk weight loads.
            identr = const_pool.tile([P, P], f32r, tag="identr")
            nc.sync.dma_start(identr[:], identr_d.ap())
            t_cur = w_pool.tile([P, DT, DZ], f32r, tag="t0")
            t0_src = t0_d.ap().rearrange("p (t b) -> p t b", t=DT)
            for kt in range(DT):
                nc.sync.dma_start(t_cur[:, kt:kt + 1, :], t0_src[:, kt:kt + 1, :])

            def load(dram, shape, tag, dtype=f32r):
                r = w_pool.tile(shape, dtype, tag=tag)
                nc.sync.dma_start(
                    r[:], dram.ap().rearrange("p (t b) -> p t b", t=shape[1]))
                return r

            xt_r = load(xtr_d, [P, DT, BC], "xtr")
            xt = load(xt_d, [P, DT, BC], "xt", dtype=f32)
            w1t = load(w1t_d, [P, DT, H], "w1t")
            w2t = load(w2t_d, [P, HT, DY], "w2t")

            b1t = const_pool.tile([P, HT], f32, tag="b1t")
            nc.sync.dma_start(b1t[:], b1t_d.ap())
            b2t = const_pool.tile([DY, 1], f32, tag="b2t")
            nc.sync.dma_start(b2t[:], b2t_d.ap())
            ident = const_pool.tile([P, P], f32, tag="ident")
            nc.sync.dma_start(ident[:], ident_d.ap())

            # PE warm-up: HAM only unthrottles (1.2 -> 2.4 GHz) after ~3.4us of
            # sustained matmul activity, and transpose-mode doesn't count.
            # Burn idle DMA-wait time on dummy matmuls so the chain starts warm.
            ps_w = psum_y_pool.tile([P, P], f32, tag="psy")
            for _ in range(28):
                nc.tensor.matmul(ps_w[:], identr[:], identr[:],
                                 start=True, stop=True)

            # D_0 = T_0^T built on device with PE transposes — runs while the
            # bulk DMAs stream and saves a 1 MiB load on the critical path.
            # (Only needed when the chain has >= 1 squaring level, i.e. n >= 4.)
            if n >= 4:
                d_cur = w_pool.tile([P, DT, DZ], f32r, tag="d0")
                for b in range(DT):
                    for a in range(DT):
                        ps = psum_pool.tile([P, P], f32r, tag="ps")
                        nc.tensor.transpose(
                            ps[:], t_cur[:, b, a * P:(a + 1) * P], identr[:])
                        nc.vector.tensor_copy(d_cur[:, a, b * P:(b + 1) * P], ps[:])
            else:
                d_cur = None

            # ---- binary exponentiation on the deviation chain --------------
            acc = xt_r          # zT accumulator, fp32r [P, DT, BC]
            acc_f32 = xt        # exact fp32 twin, used for the fused +acc add

            def apply_T(t_tile, acc_r, acc_exact):
                """acc <- acc + D @ acc   (W^(2^k) application)."""
                new_r = acc_pool.tile([P, DT, BC], f32r, tag="acc")

                def evict(mt, ps):
                    nc.vector.scalar_tensor_tensor(
                        new_r[:, mt, :], acc_exact[:, mt, :], 1.0, ps[:],
                        op0=mult, op1=add)

                _emit_mm_set(nc, psum_pool, t_tile, acc_r, evict)
                return new_r, new_r

            def square_level(d_tile, t_tile, with_d):
                """One chain level: T' = 2T + T@T (and D' = 2D + D@D when
                still needed). T and D sets interleave per output tile mt so
                that evictions for k-tile kt land early — the next level's
                MM(mt, kt) only needs the kt-th evictions, so levels overlap
                with no PE bubble."""
                t_new = chain_pool.tile([P, DT, DZ], f32r, tag="T")
                if with_d:
                    d_new = chain_pool.tile([P, DT, DZ], f32r, tag="D")
                else:
                    d_new = None
                for mt in range(DT):
                    ps_t = psum_pool.tile([P, BC], f32, tag="ps")
                    for kt in range(DT):
                        nc.tensor.matmul(
                            ps_t[:], d_tile[:, kt, mt * P:(mt + 1) * P],
                            t_tile[:, kt, :], start=(kt == 0), stop=(kt == DT - 1))
                    nc.vector.scalar_tensor_tensor(
                        t_new[:, mt, :], t_tile[:, mt, :], 2.0, ps_t[:],
                        op0=mult, op1=add)
                    if with_d:
                        ps_d = psum_pool.tile([P, BC], f32, tag="ps")
                        for kt in range(DT):
                            nc.tensor.matmul(
                                ps_d[:], t_tile[:, kt, mt * P:(mt + 1) * P],
                                d_tile[:, kt, :], start=(kt == 0), stop=(kt == DT - 1))
                        nc.vector.scalar_tensor_tensor(
                            d_new[:, mt, :], d_tile[:, mt, :], 2.0, ps_d[:],
                            op0=mult, op1=add)
                return t_new, d_new

            if n == 1:
                acc, acc_f32 = apply_T(t_cur, acc, acc_f32)
            elif n > 1:
                # Binary exponentiation; the top bit is applied as a *fused
                # double application* of T_{mb-1}:
                #   u = T^t @ acc ; z = acc + 2u + T^t @ u
                # which skips the last chain level entirely (T_mb and D_{mb-1}
                # sets are never built): 32 fewer matmuls at ~no accuracy cost.
                mb = n.bit_length() - 1
                for k in range(0, mb):
                    if (n >> k) & 1:
                        acc, acc_f32 = apply_T(t_cur, acc, acc_f32)
                    if k < mb - 1:
                        t_cur, d_cur = square_level(d_cur, t_cur,
                                                    with_d=(k + 1 < mb - 1))
                u = acc_pool.tile([P, DT, BC], f32r, tag="uacc")
                pre = acc_pool.tile([P, DT, BC], f32, tag="upre")
                for mt in range(DT):
                    ps = psum_pool.tile([P, BC], f32, tag="ps")
                    for kt in range(DT):
                        nc.tensor.matmul(
                            ps[:], t_cur[:, kt, mt * P:(mt + 1) * P],
                            acc[:, kt, :], start=(kt == 0), stop=(kt == DT - 1))
                    nc.scalar.activation(
                        u[:, mt, :], ps[:], mybir.ActivationFunctionType.Copy)
                    # pre = acc + 2u, off the critical path (overlaps the
                    # second matmul set); the final evict is then a single op.
                    nc.vector.scalar_tensor_tensor(
                        pre[:, mt, :], u[:, mt, :], 2.0, acc_f32[:, mt, :],
                        op0=mult, op1=add)
                znew = acc_pool.tile([P, DT, BC], f32r, tag="acc")
                for mt in range(DT):
                    ps = psum_pool.tile([P, BC], f32, tag="ps")
                    for kt in range(DT):
                        nc.tensor.matmul(
                            ps[:], t_cur[:, kt, mt * P:(mt + 1) * P],
                            u[:, kt, :], start=(kt == 0), stop=(kt == DT - 1))
                    nc.vector.scalar_tensor_tensor(
                        znew[:, mt, :], pre[:, mt, :], 1.0, ps[:],
                        op0=mult, op1=add)
                acc = znew

            zt = acc  # fp32r [P, DT, BC]

            # ---- MLP: hT = relu(W1 @ z + b1); yT = W2 @ h + b2 -------------
            # Layer-2 accumulation MMs interleave with layer-1 so the tail
            # after the last h-tile is just one MM + bias + transpose.
            ht = act_pool.tile([P, HT, BC], f32r, tag="ht")
            ps_y = psum_y_pool.tile([DY, BC], f32, tag="psy")
            for mt in range(HT):
                ps = psum_pool.tile([P, BC], f32, tag="ps")
                for kt in range(DT):
                    nc.tensor.matmul(
                        ps[:], w1t[:, kt, mt * P:(mt + 1) * P], zt[:, kt, :],
                        start=(kt == 0), stop=(kt == DT - 1))
                nc.scalar.activation(
                    ht[:, mt, :], ps[:], mybir.ActivationFunctionType.Relu,
                    bias=b1t[:, mt:mt + 1])
                nc.tensor.matmul(ps_y[:], w2t[:, mt, :], ht[:, mt, :],
                                 start=(mt == 0), stop=(mt == HT - 1))
            ytb = out_pool.tile([DY, BC], f32, tag="ytb")
            nc.scalar.activation(ytb[:], ps_y[:],
                                 mybir.ActivationFunctionType.Identity,
                                 bias=b2t[:])

            # ---- transpose yT -> y and store -------------------------------
            y_sb = out_pool.tile([P, BC // P, DY], f32, tag="ysb")
            for bt in range(BC // P):
                ps_t = psum_y_pool.tile([P, DY], f32, tag="psy")
                nc.tensor.transpose(
                    ps_t[:], ytb[:, bt * P:(bt + 1) * P], ident[:DY, :DY])
                nc.vector.tensor_copy(y_sb[:, bt, :], ps_t[:])
            nc.sync.dma_start(
                y_d.ap().rearrange("(bt p) j -> p bt j", p=P), y_sb[:])

    nc.compile()
    return nc


def _tiles_pk(m: np.ndarray) -> np.ndarray:
    """[nt*128, C] -> [128, nt*C] partition-tiled layout (row r = kt*128+p)."""
    nt = m.shape[0] // P
    return np.ascontiguousarray(m.reshape(nt, P, -1).swapaxes(0, 1)).reshape(P, -1)


def kernel(x, A, W1, b1, W2, b2, n_steps) -> np.ndarray:
    x = np.asarray(x, dtype=np.float32)
    A = np.asarray(A, dtype=np.float32)
    W1 = np.asarray(W1, dtype=np.float32)
    b1 = np.asarray(b1, dtype=np.float32)
    W2 = np.asarray(W2, dtype=np.float32)
    b2 = np.asarray(b2, dtype=np.float32)
    n = int(np.asarray(n_steps))

    if n not in _BUILD_CACHE:
        _BUILD_CACHE[n] = _build(n)
    nc = _BUILD_CACHE[n]

    dt = np.float32(1.0 / n) if n > 0 else np.float32(0.0)
    t0 = _tiles_pk(np.ascontiguousarray(dt * A.T, dtype=np.float32))
    w1t = _tiles_pk(np.ascontiguousarray(W1.T))           # [512, 2048]
    w2t = _tiles_pk(np.ascontiguousarray(W2.T))           # [2048, 10]
    b1t = np.ascontiguousarray(b1.reshape(HT, P).T)       # [128, 16]
    b2t = np.ascontiguousarray(b2.reshape(DY, 1))
    ident = np.eye(P, dtype=np.float32)

    in_maps = []
    for c in range(NCORES):
        xs = x[c * BC:(c + 1) * BC, :]                    # [512, 512]
        xt = _tiles_pk(np.ascontiguousarray(xs.T))        # [128, 4*512]
        in_maps.append({
            "xt": xt, "xtr": xt, "t0": t0, "w1t": w1t, "b1t": b1t,
            "w2t": w2t, "b2t": b2t, "ident": ident, "identr": ident,
        })

    trace = bool(os.environ.get("BASS_KERNEL_TRACE"))
    kwargs = {}
    if trace:
        kwargs = {"trace": True, "trace_cores": [0]}
    res = run_bass_kernel_spmd(nc, in_maps, list(range(NCORES)), **kwargs)
    if trace and res.exec_time_ns is not None:
        print(f"HW exec time: {res.exec_time_ns} ns")

    y = np.concatenate([res.results[c]["y"] for c in range(NCORES)], axis=0)
    return y.astype(np.float32)


# revision 28
# speedup vs baseline: 1.0026x; 1.0026x over previous
"""Trainium2 Bass kernel for MatrixOdeGradientDescentModel.

Reference computation (B=4096, DZ=512, H=2048, DY=10, n_steps=64):
    z = x; repeat n_steps: z += dt * z @ A.T          (dt = 1/n_steps)
    y = relu(z @ W1.T + b1) @ W2.T + b2

Algebraic rewrite: the Euler loop is linear, so
    z_final = x @ (M^T)^n  with  M = I + dt*A.
We compute W := M^T = I + dt*A^T by repeated squaring on the *deviation*
D_k := W^(2^k) - I (avoids precision loss from the identity's magnitude):
    D_{k+1} = 2*D_k + D_k @ D_k
maintaining the pair (D_k, T_k=D_k^T) so no on-device transposes are needed:
    D@D = matmul(lhsT=T, rhs=D),   (D@D)^T = matmul(lhsT=D, rhs=T)
then zT = xT + D_chain applied to xT per set bit of n (binary exponentiation).

Sharding: data-parallel over batch. Each of the 8 cores gets 512 rows of x;
A/W1/W2 replicated; no cross-core communication.

Matmuls run in float32r (TF32-like, 4x faster than fp32 on the PE) with fp32
PSUM accumulation; the error-compensated deviation chain keeps the end-to-end
relative error at the ~1e-4 level.
"""

import os

import numpy as np

import concourse.bacc as bacc
import concourse.mybir as mybir
import concourse.tile as tile
from concourse.bass_utils import run_bass_kernel_spmd

P = 128
B, DZ, H, DY = 4096, 512, 2048, 10
NCORES = 8
BC = B // NCORES          # 512 rows per core
DT = DZ // P              # 4 k-tiles over DZ
HT = H // P               # 16 m-tiles over H

f32 = mybir.dt.float32
f32r = mybir.dt.float32r

_BUILD_CACHE = {}


def _emit_mm_set(nc, psum_pool, lhsT_tile, rhs_tile, evict, n_mt=DT):
    """One [512,512]-ish matmul set: for each output row-block mt, accumulate
    over DT k-tiles into PSUM and call evict(mt, psum_ap)."""
    for mt in range(n_mt):
        ps = psum_pool.tile([P, BC], f32, tag="ps")
        for kt in range(DT):
            nc.tensor.matmul(
                ps[:],
                lhsT_tile[:, kt, mt * P:(mt + 1) * P],
                rhs_tile[:, kt, :],
                start=(kt == 0),
                stop=(kt == DT - 1),
            )
        evict(mt, ps)


def _build(n_steps: int):
    """Build + compile the Bass module for a given n_steps."""
    n = int(n_steps)
    assert n >= 0
    nc = bacc.Bacc("TRN2", target_bir_lowering=False, debug=False,
                   enable_asserts=False, num_devices=NCORES)

    # f32r-declared DRAM inputs carry raw fp32 bytes; the PE rounds internally
    # (verified bit-identical to an explicit cast) so plain HWDGE DMA works.
    xt_d = nc.dram_tensor("xt", [P, DT * BC], f32, kind="ExternalInput")
    xtr_d = nc.dram_tensor("xtr", [P, DT * BC], f32r, kind="ExternalInput")
    t0_d = nc.dram_tensor("t0", [P, DT * DZ], f32r, kind="ExternalInput")
    w1t_d = nc.dram_tensor("w1t", [P, DT * H], f32r, kind="ExternalInput")
    b1t_d = nc.dram_tensor("b1t", [P, HT], f32, kind="ExternalInput")
    w2t_d = nc.dram_tensor("w2t", [P, HT * DY], f32r, kind="ExternalInput")
    b2t_d = nc.dram_tensor("b2t", [DY, 1], f32, kind="ExternalInput")
    ident_d = nc.dram_tensor("ident", [P, P], f32, kind="ExternalInput")
    identr_d = nc.dram_tensor("identr", [P, P], f32r, kind="ExternalInput")
    y_d = nc.dram_tensor("y", [BC, DY], f32, kind="ExternalOutput")

    mult = mybir.AluOpType.mult
    add = mybir.AluOpType.add

    with tile.TileContext(nc) as tc:
        with (
            tc.tile_pool(name="const", bufs=1) as const_pool,
            tc.tile_pool(name="weights", bufs=1) as w_pool,
            tc.tile_pool(name="chain", bufs=2) as chain_pool,
            tc.tile_pool(name="accp", bufs=2) as acc_pool,
            tc.tile_pool(name="acts", bufs=1) as act_pool,
            tc.tile_pool(name="out", bufs=2) as out_pool,
            tc.tile_pool(name="psum", bufs=7, space="PSUM") as psum_pool,
            tc.tile_pool(name="psum_y", bufs=1, space="PSUM") as psum_y_pool,
        ):
            # ---- loads (all fast HWDGE; chain inputs first) ----------------
            def load(dram, shape, tag, dtype=f32r, chunks=1):
                r = w_pool.tile(shape, dtype, tag=tag)
                src = dram.ap().rearrange("p (t b) -> p t b", t=shape[1])
                for ch in range(chunks):
                    lo = shape[1] * ch // chunks
                    hi = shape[1] * (ch + 1) // chunks
                    nc.sync.dma_start(r[:, lo:hi, :], src[:, lo:hi, :])
                return r

            # All loads go through one trigger queue (Sync) in priority order:
            # the DMA rings are FIFO, so t0 — which gates the squaring
            # chain — must be enqueued before the bulk weight loads.
            identr = const_pool.tile([P, P], f32r, tag="identr")
            nc.sync.dma_start(identr[:], identr_d.ap())
            t_cur = w_pool.tile([P, DT, DZ], f32r, tag="t0")
            t0_src = t0_d.ap().rearrange("p (t b) -> p t b", t=DT)
            for kt in range(DT):
                nc.sync.dma_start(t_cur[:, kt:kt + 1, :], t0_src[:, kt:kt + 1, :])

            def load(dram, shape, tag, dtype=f32r):
                r = w_pool.tile(shape, dtype, tag=tag)
                nc.sync.dma_start(
                    r[:], dram.ap().rearrange("p (t b) -> p t b", t=shape[1]))
                return r

            xt_r = load(xtr_d, [P, DT, BC], "xtr")
            xt = load(xt_d, [P, DT, BC], "xt", dtype=f32)
            w1t = load(w1t_d, [P, DT, H], "w1t")
            w2t = load(w2t_d, [P, HT, DY], "w2t")

            b1t = const_pool.tile([P, HT], f32, tag="b1t")
            nc.sync.dma_start(b1t[:], b1t_d.ap())
            b2t = const_pool.tile([DY, 1], f32, tag="b2t")
            nc.sync.dma_start(b2t[:], b2t_d.ap())
            ident = const_pool.tile([P, P], f32, tag="ident")
            nc.sync.dma_start(ident[:], ident_d.ap())

            # Brief PE warm-up while waiting for t0: HAM only unthrottles
            # (1.2 -> 2.4 GHz) after ~3.4us of sustained matmul activity, so
            # start the activity window as early as possible. Alternate two
            # PSUM tiles so the dummies pipeline instead of WAW-serializing.
            ps_w0 = psum_y_pool.tile([P, P], f32, tag="psy")
            ps_w1 = psum_pool.tile([P, P], f32, tag="ps")
            for i in range(6):
                nc.tensor.matmul([ps_w0, ps_w1][i % 2][:], identr[:], identr[:],
                                 start=True, stop=True)

            # D_0 = T_0^T built on device via matmul against the identity
            # (a regular matmul, not transpose-mode, so it also counts toward
            # HAM warm-up). Runs while the bulk DMAs stream and saves a 1 MiB
            # load on the critical path. Only needed when the chain has >= 1
            # squaring level, i.e. n >= 4.
            if n >= 4:
                d_cur = w_pool.tile([P, DT, DZ], f32r, tag="d0")
                for b in range(DT):
                    for a in range(DT):
                        ps = psum_pool.tile([P, P], f32, tag="ps")
                        nc.tensor.matmul(
                            ps[:], t_cur[:, b, a * P:(a + 1) * P], identr[:],
                            start=True, stop=True)
                        nc.vector.tensor_copy(d_cur[:, a, b * P:(b + 1) * P], ps[:])
            else:
                d_cur = None

            # ---- binary exponentiation on the deviation chain --------------
            acc = xt_r          # zT accumulator, fp32r [P, DT, BC]
            acc_f32 = xt        # exact fp32 twin, used for the fused +acc add

            def apply_T(t_tile, acc_r, acc_exact):
                """acc <- acc + D @ acc   (W^(2^k) application)."""
                new_r = acc_pool.tile([P, DT, BC], f32r, tag="acc")

                def evict(mt, ps):
                    nc.vector.scalar_tensor_tensor(
                        new_r[:, mt, :], acc_exact[:, mt, :], 1.0, ps[:],
                        op0=mult, op1=add)

                _emit_mm_set(nc, psum_pool, t_tile, acc_r, evict)
                return new_r, new_r

            def square_level(d_tile, t_tile, with_d):
                """One chain level: T' = 2T + T@T (and D' = 2D + D@D when
                still needed). T and D sets interleave per output tile mt so
                that evictions for k-tile kt land early — the next level's
                MM(mt, kt) only needs the kt-th evictions, so levels overlap
                with no PE bubble."""
                t_new = chain_pool.tile([P, DT, DZ], f32r, tag="T")
                if with_d:
                    d_new = chain_pool.tile([P, DT, DZ], f32r, tag="D")
                else:
                    d_new = None
                for mt in range(DT):
                    ps_t = psum_pool.tile([P, BC], f32, tag="ps")
                    for kt in range(DT):
                        nc.tensor.matmul(
                            ps_t[:], d_tile[:, kt, mt * P:(mt + 1) * P],
                            t_tile[:, kt, :], start=(kt == 0), stop=(kt == DT - 1))
                    nc.vector.scalar_tensor_tensor(
                        t_new[:, mt, :], t_tile[:, mt, :], 2.0, ps_t[:],
                        op0=mult, op1=add)
                    if with_d:
                        ps_d = psum_pool.tile([P, BC], f32, tag="ps")
                        for kt in range(DT):
                            nc.tensor.matmul(
                                ps_d[:], t_tile[:, kt, mt * P:(mt + 1) * P],
                                d_tile[:, kt, :], start=(kt == 0), stop=(kt == DT - 1))
                        nc.vector.scalar_tensor_tensor(
                            d_new[:, mt, :], d_tile[:, mt, :], 2.0, ps_d[:],
                            op0=mult, op1=add)
                return t_new, d_new

            if n == 1:
                acc, acc_f32 = apply_T(t_cur, acc, acc_f32)
            elif n > 1:
                # Binary exponentiation; the top bit is applied as a *fused
                # double application* of T_{mb-1}:
                #   u = T^t @ acc ; z = acc + 2u + T^t @ u
                # which skips the last chain level entirely (T_mb and D_{mb-1}
                # sets are never built): 32 fewer matmuls at ~no accuracy cost.
                mb = n.bit_length() - 1
                for k in range(0, mb):
                    if (n >> k) & 1:
                        acc, acc_f32 = apply_T(t_cur, acc, acc_f32)
                    if k < mb - 1:
                        t_cur, d_cur = square_level(d_cur, t_cur,
                                                    with_d=(k + 1 < mb - 1))
                u = acc_pool.tile([P, DT, BC], f32r, tag="uacc")
                pre = acc_pool.tile([P, DT, BC], f32, tag="upre")
                for mt in range(DT):
                    ps = psum_pool.tile([P, BC], f32, tag="ps")
                    for kt in range(DT):
                        nc.tensor.matmul(
                            ps[:], t_cur[:, kt, mt * P:(mt + 1) * P],
                            acc[:, kt, :], start=(kt == 0), stop=(kt == DT - 1))
                    nc.scalar.activation(
                        u[:, mt, :], ps[:], mybir.ActivationFunctionType.Copy)
                    # pre = acc + 2u, off the critical path (overlaps the
                    # second matmul set); the final evict is then a single op.
                    nc.vector.scalar_tensor_tensor(
                        pre[:, mt, :], u[:, mt, :], 2.0, acc_f32[:, mt, :],
                        op0=mult, op1=add)
                znew = acc_pool.tile([P, DT, BC], f32r, tag="acc")
                for mt in range(DT):
                    ps = psum_pool.tile([P, BC], f32, tag="ps")
                    for kt in range(DT):
                        nc.tensor.matmul(
                            ps[:], t_cur[:, kt, mt * P:(mt + 1) * P],
                            u[:, kt, :], start=(kt == 0), stop=(kt == DT - 1))
                    nc.vector.scalar_tensor_tensor(
                        znew[:, mt, :], pre[:, mt, :], 1.0, ps[:],
                        op0=mult, op1=add)
                acc = znew

            zt = acc  # fp32r [P, DT, BC]

            # ---- MLP: hT = relu(W1 @ z + b1); yT = W2 @ h + b2 -------------
            # Layer-2 accumulation MMs interleave with layer-1 so the tail
            # after the last h-tile is just one MM + bias + transpose.
            ht = act_pool.tile([P, HT, BC], f32r, tag="ht")
            ps_y = psum_y_pool.tile([DY, BC], f32, tag="psy")
            for mt in range(HT):
                ps = psum_pool.tile([P, BC], f32, tag="ps")
                for kt in range(DT):
                    nc.tensor.matmul(
                        ps[:], w1t[:, kt, mt * P:(mt + 1) * P], zt[:, kt, :],
                        start=(kt == 0), stop=(kt == DT - 1))
                nc.scalar.activation(
                    ht[:, mt, :], ps[:], mybir.ActivationFunctionType.Relu,
                    bias=b1t[:, mt:mt + 1])
                nc.tensor.matmul(ps_y[:], w2t[:, mt, :], ht[:, mt, :],
                                 start=(mt == 0), stop=(mt == HT - 1))
            ytb = out_pool.tile([DY, BC], f32, tag="ytb")
            nc.scalar.activation(ytb[:], ps_y[:],
                                 mybir.ActivationFunctionType.Identity,
                                 bias=b2t[:])

            # ---- transpose yT -> y and store -------------------------------
            y_sb = out_pool.tile([P, BC // P, DY], f32, tag="ysb")
            for bt in range(BC // P):
                ps_t = psum_y_pool.tile([P, DY], f32, tag="psy")
                nc.tensor.transpose(
                    ps_t[:], ytb[:, bt * P:(bt + 1) * P], ident[:DY, :DY])
                nc.vector.tensor_copy(y_sb[:, bt, :], ps_t[:])
            nc.sync.dma_start(
                y_d.ap().rearrange("(bt p) j -> p bt j", p=P), y_sb[:])

    nc.compile()
    return nc


def _tiles_pk(m: np.ndarray) -> np.ndarray:
    """[nt*128, C] -> [128, nt*C] partition-tiled layout (row r = kt*128+p)."""
    nt = m.shape[0] // P
    return np.ascontiguousarray(m.reshape(nt, P, -1).swapaxes(0, 1)).reshape(P, -1)


def kernel(x, A, W1, b1, W2, b2, n_steps) -> np.ndarray:
    x = np.asarray(x, dtype=np.float32)
    A = np.asarray(A, dtype=np.float32)
    W1 = np.asarray(W1, dtype=np.float32)
    b1 = np.asarray(b1, dtype=np.float32)
    W2 = np.asarray(W2, dtype=np.float32)
    b2 = np.asarray(b2, dtype=np.float32)
    n = int(np.asarray(n_steps))

    if n not in _BUILD_CACHE:
        _BUILD_CACHE[n] = _build(n)
    nc = _BUILD_CACHE[n]

    dt = np.float32(1.0 / n) if n > 0 else np.float32(0.0)
    t0 = _tiles_pk(np.ascontiguousarray(dt * A.T, dtype=np.float32))
    w1t = _tiles_pk(np.ascontiguousarray(W1.T))           # [512, 2048]
    w2t = _tiles_pk(np.ascontiguousarray(W2.T))           # [2048, 10]
    b1t = np.ascontiguousarray(b1.reshape(HT, P).T)       # [128, 16]
    b2t = np.ascontiguousarray(b2.reshape(DY, 1))
    ident = np.eye(P, dtype=np.float32)

    in_maps = []
    for c in range(NCORES):
        xs = x[c * BC:(c + 1) * BC, :]                    # [512, 512]
        xt = _tiles_pk(np.ascontiguousarray(xs.T))        # [128, 4*512]
        in_maps.append({
            "xt": xt, "xtr": xt, "t0": t0, "w1t": w1t, "b1t": b1t,
            "w2t": w2t, "b2t": b2t, "ident": ident, "identr": ident,
        })

    trace = bool(os.environ.get("BASS_KERNEL_TRACE"))
    kwargs = {}
    if trace:
        kwargs = {"trace": True, "trace_cores": [0]}
    res = run_bass_kernel_spmd(nc, in_maps, list(range(NCORES)), **kwargs)
    if trace and res.exec_time_ns is not None:
        print(f"HW exec time: {res.exec_time_ns} ns")

    y = np.concatenate([res.results[c]["y"] for c in range(NCORES)], axis=0)
    return y.astype(np.float32)


# revision 29
# speedup vs baseline: 1.0171x; 1.0145x over previous
"""Trainium2 Bass kernel for MatrixOdeGradientDescentModel.

Reference computation (B=4096, DZ=512, H=2048, DY=10, n_steps=64):
    z = x; repeat n_steps: z += dt * z @ A.T          (dt = 1/n_steps)
    y = relu(z @ W1.T + b1) @ W2.T + b2

Algebraic rewrite: the Euler loop is linear, so
    z_final = x @ (M^T)^n  with  M = I + dt*A.
We compute W := M^T = I + dt*A^T by repeated squaring on the *deviation*
D_k := W^(2^k) - I (avoids precision loss from the identity's magnitude):
    D_{k+1} = 2*D_k + D_k @ D_k
maintaining the pair (D_k, T_k=D_k^T) so no on-device transposes are needed:
    D@D = matmul(lhsT=T, rhs=D),   (D@D)^T = matmul(lhsT=D, rhs=T)
then zT = xT + D_chain applied to xT per set bit of n (binary exponentiation).

Sharding: data-parallel over batch. Each of the 8 cores gets 512 rows of x;
A/W1/W2 replicated; no cross-core communication.

Matmuls run in float32r (TF32-like, 4x faster than fp32 on the PE) with fp32
PSUM accumulation; the error-compensated deviation chain keeps the end-to-end
relative error at the ~1e-4 level.
"""

import os

import numpy as np

import concourse.bacc as bacc
import concourse.mybir as mybir
import concourse.tile as tile
from concourse.bass_utils import run_bass_kernel_spmd

P = 128
B, DZ, H, DY = 4096, 512, 2048, 10
NCORES = 8
BC = B // NCORES          # 512 rows per core
DT = DZ // P              # 4 k-tiles over DZ
HT = H // P               # 16 m-tiles over H

f32 = mybir.dt.float32
f32r = mybir.dt.float32r

_BUILD_CACHE = {}


def _emit_mm_set(nc, psum_pool, lhsT_tile, rhs_tile, evict, n_mt=DT):
    """One [512,512]-ish matmul set: for each output row-block mt, accumulate
    over DT k-tiles into PSUM and call evict(mt, psum_ap)."""
    for mt in range(n_mt):
        ps = psum_pool.tile([P, BC], f32, tag="ps")
        for kt in range(DT):
            nc.tensor.matmul(
                ps[:],
                lhsT_tile[:, kt, mt * P:(mt + 1) * P],
                rhs_tile[:, kt, :],
                start=(kt == 0),
                stop=(kt == DT - 1),
            )
        evict(mt, ps)


def _build(n_steps: int):
    """Build + compile the Bass module for a given n_steps."""
    n = int(n_steps)
    assert n >= 0
    nc = bacc.Bacc("TRN2", target_bir_lowering=False, debug=False,
                   enable_asserts=False, num_devices=NCORES)

    # f32r-declared DRAM inputs carry raw fp32 bytes; the PE rounds internally
    # (verified bit-identical to an explicit cast) so plain HWDGE DMA works.
    xt_d = nc.dram_tensor("xt", [P, DT * BC], f32, kind="ExternalInput")
    xtr_d = nc.dram_tensor("xtr", [P, DT * BC], f32r, kind="ExternalInput")
    t0_d = nc.dram_tensor("t0", [P, DT * DZ], f32r, kind="ExternalInput")
    w1t_d = nc.dram_tensor("w1t", [P, DT * H], f32r, kind="ExternalInput")
    b1t_d = nc.dram_tensor("b1t", [P, HT], f32, kind="ExternalInput")
    w2t_d = nc.dram_tensor("w2t", [P, HT * DY], f32r, kind="ExternalInput")
    b2t_d = nc.dram_tensor("b2t", [DY, 1], f32, kind="ExternalInput")
    ident_d = nc.dram_tensor("ident", [P, P], f32, kind="ExternalInput")
    identr_d = nc.dram_tensor("identr", [P, P], f32r, kind="ExternalInput")
    y_d = nc.dram_tensor("y", [BC, DY], f32, kind="ExternalOutput")

    mult = mybir.AluOpType.mult
    add = mybir.AluOpType.add

    with tile.TileContext(nc) as tc:
        with (
            tc.tile_pool(name="const", bufs=1) as const_pool,
            tc.tile_pool(name="weights", bufs=1) as w_pool,
            tc.tile_pool(name="chain", bufs=2) as chain_pool,
            tc.tile_pool(name="accp", bufs=2) as acc_pool,
            tc.tile_pool(name="acts", bufs=1) as act_pool,
            tc.tile_pool(name="out", bufs=2) as out_pool,
            tc.tile_pool(name="psum", bufs=7, space="PSUM") as psum_pool,
            tc.tile_pool(name="psum_y", bufs=1, space="PSUM") as psum_y_pool,
        ):
            # ---- loads (all fast HWDGE; chain inputs first) ----------------
            def load(dram, shape, tag, dtype=f32r, chunks=1):
                r = w_pool.tile(shape, dtype, tag=tag)
                src = dram.ap().rearrange("p (t b) -> p t b", t=shape[1])
                for ch in range(chunks):
                    lo = shape[1] * ch // chunks
                    hi = shape[1] * (ch + 1) // chunks
                    nc.sync.dma_start(r[:, lo:hi, :], src[:, lo:hi, :])
                return r

            # All loads go through one trigger queue (Sync) in priority order:
            # the DMA rings are FIFO, so t0 — which gates the squaring
            # chain — must be enqueued before the bulk weight loads.
            identr = const_pool.tile([P, P], f32r, tag="identr")
            nc.sync.dma_start(identr[:], identr_d.ap())
            t_cur = w_pool.tile([P, DT, DZ], f32r, tag="t0")
            t0_src = t0_d.ap().rearrange("p (t b) -> p t b", t=DT)
            for kt in range(DT):
                nc.sync.dma_start(t_cur[:, kt:kt + 1, :], t0_src[:, kt:kt + 1, :])

            def load(dram, shape, tag, dtype=f32r):
                r = w_pool.tile(shape, dtype, tag=tag)
                nc.sync.dma_start(
                    r[:], dram.ap().rearrange("p (t b) -> p t b", t=shape[1]))
                return r

            xt_r = load(xtr_d, [P, DT, BC], "xtr")
            xt = load(xt_d, [P, DT, BC], "xt", dtype=f32)
            w1t = load(w1t_d, [P, DT, H], "w1t")
            w2t = load(w2t_d, [P, HT, DY], "w2t")

            b1t = const_pool.tile([P, HT], f32, tag="b1t")
            nc.sync.dma_start(b1t[:], b1t_d.ap())
            b2t = const_pool.tile([DY, 1], f32, tag="b2t")
            nc.sync.dma_start(b2t[:], b2t_d.ap())
            ident = const_pool.tile([P, P], f32, tag="ident")
            nc.sync.dma_start(ident[:], ident_d.ap())

            # Brief PE warm-up while waiting for t0: HAM only unthrottles
            # (1.2 -> 2.4 GHz) after ~3.4us of sustained matmul activity, so
            # start the activity window as early as possible. Alternate two
            # PSUM tiles so the dummies pipeline instead of WAW-serializing.
            ps_w0 = psum_y_pool.tile([P, P], f32, tag="psy")
            ps_w1 = psum_pool.tile([P, P], f32, tag="ps")
            for i in range(6):
                nc.tensor.matmul([ps_w0, ps_w1][i % 2][:], identr[:], identr[:],
                                 start=True, stop=True)

            # D_0 = T_0^T built on device via matmul against the identity
            # (a regular matmul, not transpose-mode, so it also counts toward
            # HAM warm-up). Runs while the bulk DMAs stream and saves a 1 MiB
            # load on the critical path. Only needed when the chain has >= 1
            # squaring level, i.e. n >= 4.
            if n >= 4:
                d_cur = w_pool.tile([P, DT, DZ], f32r, tag="d0")
                for a in range(DT):
                    ps = psum_pool.tile([P, DZ], f32, tag="ps")
                    for b in range(DT):
                        nc.tensor.matmul(
                            ps[:, b * P:(b + 1) * P],
                            t_cur[:, b, a * P:(a + 1) * P], identr[:],
                            start=True, stop=True)
                    nc.vector.tensor_copy(d_cur[:, a, :], ps[:])
            else:
                d_cur = None

            # ---- binary exponentiation on the deviation chain --------------
            acc = xt_r          # zT accumulator, fp32r [P, DT, BC]
            acc_f32 = xt        # exact fp32 twin, used for the fused +acc add

            def apply_T(t_tile, acc_r, acc_exact):
                """acc <- acc + D @ acc   (W^(2^k) application)."""
                new_r = acc_pool.tile([P, DT, BC], f32r, tag="acc")

                def evict(mt, ps):
                    nc.vector.scalar_tensor_tensor(
                        new_r[:, mt, :], acc_exact[:, mt, :], 1.0, ps[:],
                        op0=mult, op1=add)

                _emit_mm_set(nc, psum_pool, t_tile, acc_r, evict)
                return new_r, new_r

            def square_level(d_tile, t_tile, with_d):
                """One chain level: T' = 2T + T@T (and D' = 2D + D@D when
                still needed). T and D sets interleave per output tile mt so
                that evictions for k-tile kt land early — the next level's
                MM(mt, kt) only needs the kt-th evictions, so levels overlap
                with no PE bubble."""
                t_new = chain_pool.tile([P, DT, DZ], f32r, tag="T")
                if with_d:
                    d_new = chain_pool.tile([P, DT, DZ], f32r, tag="D")
                else:
                    d_new = None
                for mt in range(DT):
                    ps_t = psum_pool.tile([P, BC], f32, tag="ps")
                    for kt in range(DT):
                        nc.tensor.matmul(
                            ps_t[:], d_tile[:, kt, mt * P:(mt + 1) * P],
                            t_tile[:, kt, :], start=(kt == 0), stop=(kt == DT - 1))
                    nc.vector.scalar_tensor_tensor(
                        t_new[:, mt, :], t_tile[:, mt, :], 2.0, ps_t[:],
                        op0=mult, op1=add)
                    if with_d:
                        ps_d = psum_pool.tile([P, BC], f32, tag="ps")
                        for kt in range(DT):
                            nc.tensor.matmul(
                                ps_d[:], t_tile[:, kt, mt * P:(mt + 1) * P],
                                d_tile[:, kt, :], start=(kt == 0), stop=(kt == DT - 1))
                        nc.vector.scalar_tensor_tensor(
                            d_new[:, mt, :], d_tile[:, mt, :], 2.0, ps_d[:],
                            op0=mult, op1=add)
                return t_new, d_new

            if n == 1:
                acc, acc_f32 = apply_T(t_cur, acc, acc_f32)
            elif n > 1:
                # Binary exponentiation; the top bit is applied as a *fused
                # double application* of T_{mb-1}:
                #   u = T^t @ acc ; z = acc + 2u + T^t @ u
                # which skips the last chain level entirely (T_mb and D_{mb-1}
                # sets are never built): 32 fewer matmuls at ~no accuracy cost.
                mb = n.bit_length() - 1
                for k in range(0, mb):
                    if (n >> k) & 1:
                        acc, acc_f32 = apply_T(t_cur, acc, acc_f32)
                    if k < mb - 1:
                        t_cur, d_cur = square_level(d_cur, t_cur,
                                                    with_d=(k + 1 < mb - 1))
                u = acc_pool.tile([P, DT, BC], f32r, tag="uacc")
                pre = acc_pool.tile([P, DT, BC], f32, tag="upre")
                for mt in range(DT):
                    ps = psum_pool.tile([P, BC], f32, tag="ps")
                    for kt in range(DT):
                        nc.tensor.matmul(
                            ps[:], t_cur[:, kt, mt * P:(mt + 1) * P],
                            acc[:, kt, :], start=(kt == 0), stop=(kt == DT - 1))
                    nc.scalar.activation(
                        u[:, mt, :], ps[:], mybir.ActivationFunctionType.Copy)
                    # pre = acc + 2u, off the critical path (overlaps the
                    # second matmul set); the final evict is then a single op.
                    nc.vector.scalar_tensor_tensor(
                        pre[:, mt, :], u[:, mt, :], 2.0, acc_f32[:, mt, :],
                        op0=mult, op1=add)
                znew = acc_pool.tile([P, DT, BC], f32r, tag="acc")
                for mt in range(DT):
                    ps = psum_pool.tile([P, BC], f32, tag="ps")
                    for kt in range(DT):
                        nc.tensor.matmul(
                            ps[:], t_cur[:, kt, mt * P:(mt + 1) * P],
                            u[:, kt, :], start=(kt == 0), stop=(kt == DT - 1))
                    nc.vector.scalar_tensor_tensor(
                        znew[:, mt, :], pre[:, mt, :], 1.0, ps[:],
                        op0=mult, op1=add)
                acc = znew

            zt = acc  # fp32r [P, DT, BC]

            # ---- MLP: hT = relu(W1 @ z + b1); yT = W2 @ h + b2 -------------
            # Layer-2 accumulation MMs interleave with layer-1 so the tail
            # after the last h-tile is just one MM + bias + transpose.
            ht = act_pool.tile([P, HT, BC], f32r, tag="ht")
            ps_y = psum_y_pool.tile([DY, BC], f32, tag="psy")
            for mt in range(HT):
                ps = psum_pool.tile([P, BC], f32, tag="ps")
                for kt in range(DT):
                    nc.tensor.matmul(
                        ps[:], w1t[:, kt, mt * P:(mt + 1) * P], zt[:, kt, :],
                        start=(kt == 0), stop=(kt == DT - 1))
                nc.scalar.activation(
                    ht[:, mt, :], ps[:], mybir.ActivationFunctionType.Relu,
                    bias=b1t[:, mt:mt + 1])
                nc.tensor.matmul(ps_y[:], w2t[:, mt, :], ht[:, mt, :],
                                 start=(mt == 0), stop=(mt == HT - 1))
            ytb = out_pool.tile([DY, BC], f32, tag="ytb")
            nc.scalar.activation(ytb[:], ps_y[:],
                                 mybir.ActivationFunctionType.Identity,
                                 bias=b2t[:])

            # ---- transpose yT -> y and store -------------------------------
            y_sb = out_pool.tile([P, BC // P, DY], f32, tag="ysb")
            for bt in range(BC // P):
                ps_t = psum_y_pool.tile([P, DY], f32, tag="psy")
                nc.tensor.transpose(
                    ps_t[:], ytb[:, bt * P:(bt + 1) * P], ident[:DY, :DY])
                nc.vector.tensor_copy(y_sb[:, bt, :], ps_t[:])
            nc.sync.dma_start(
                y_d.ap().rearrange("(bt p) j -> p bt j", p=P), y_sb[:])

    nc.compile()
    return nc


def _tiles_pk(m: np.ndarray) -> np.ndarray:
    """[nt*128, C] -> [128, nt*C] partition-tiled layout (row r = kt*128+p)."""
    nt = m.shape[0] // P
    return np.ascontiguousarray(m.reshape(nt, P, -1).swapaxes(0, 1)).reshape(P, -1)


def kernel(x, A, W1, b1, W2, b2, n_steps) -> np.ndarray:
    x = np.asarray(x, dtype=np.float32)
    A = np.asarray(A, dtype=np.float32)
    W1 = np.asarray(W1, dtype=np.float32)
    b1 = np.asarray(b1, dtype=np.float32)
    W2 = np.asarray(W2, dtype=np.float32)
    b2 = np.asarray(b2, dtype=np.float32)
    n = int(np.asarray(n_steps))

    if n not in _BUILD_CACHE:
        _BUILD_CACHE[n] = _build(n)
    nc = _BUILD_CACHE[n]

    dt = np.float32(1.0 / n) if n > 0 else np.float32(0.0)
    t0 = _tiles_pk(np.ascontiguousarray(dt * A.T, dtype=np.float32))
    w1t = _tiles_pk(np.ascontiguousarray(W1.T))           # [512, 2048]
    w2t = _tiles_pk(np.ascontiguousarray(W2.T))           # [2048, 10]
    b1t = np.ascontiguousarray(b1.reshape(HT, P).T)       # [128, 16]
    b2t = np.ascontiguousarray(b2.reshape(DY, 1))
    ident = np.eye(P, dtype=np.float32)

    in_maps = []
    for c in range(NCORES):
        xs = x[c * BC:(c + 1) * BC, :]                    # [512, 512]
        xt = _tiles_pk(np.ascontiguousarray(xs.T))        # [128, 4*512]
        in_maps.append({
            "xt": xt, "xtr": xt, "t0": t0, "w1t": w1t, "b1t": b1t,
            "w2t": w2t, "b2t": b2t, "ident": ident, "identr": ident,
        })

    trace = bool(os.environ.get("BASS_KERNEL_TRACE"))
    kwargs = {}
    if trace:
        kwargs = {"trace": True, "trace_cores": [0]}
    res = run_bass_kernel_spmd(nc, in_maps, list(range(NCORES)), **kwargs)
    if trace and res.exec_time_ns is not None:
        print(f"HW exec time: {res.exec_time_ns} ns")

    y = np.concatenate([res.results[c]["y"] for c in range(NCORES)], axis=0)
    return y.astype(np.float32)


# revision 33
# speedup vs baseline: 1.0197x; 1.0025x over previous
"""Trainium2 Bass kernel for MatrixOdeGradientDescentModel.

Reference computation (B=4096, DZ=512, H=2048, DY=10, n_steps=64):
    z = x; repeat n_steps: z += dt * z @ A.T          (dt = 1/n_steps)
    y = relu(z @ W1.T + b1) @ W2.T + b2

Algebraic rewrite: the Euler loop is linear, so
    z_final = x @ (M^T)^n  with  M = I + dt*A.
We compute W := M^T = I + dt*A^T by repeated squaring on the *deviation*
D_k := W^(2^k) - I (avoids precision loss from the identity's magnitude):
    D_{k+1} = 2*D_k + D_k @ D_k
maintaining the pair (D_k, T_k=D_k^T) so no on-device transposes are needed:
    D@D = matmul(lhsT=T, rhs=D),   (D@D)^T = matmul(lhsT=D, rhs=T)
then zT = xT + D_chain applied to xT per set bit of n (binary exponentiation).

Sharding: data-parallel over batch. Each of the 8 cores gets 512 rows of x;
A/W1/W2 replicated; no cross-core communication.

Matmuls run in float32r (TF32-like, 4x faster than fp32 on the PE) with fp32
PSUM accumulation; the error-compensated deviation chain keeps the end-to-end
relative error at the ~1e-4 level.
"""

import os

import numpy as np

import concourse.bacc as bacc
import concourse.mybir as mybir
import concourse.tile as tile
from concourse.bass_utils import run_bass_kernel_spmd

P = 128
B, DZ, H, DY = 4096, 512, 2048, 10
NCORES = 8
BC = B // NCORES          # 512 rows per core
DT = DZ // P              # 4 k-tiles over DZ
HT = H // P               # 16 m-tiles over H

f32 = mybir.dt.float32
f32r = mybir.dt.float32r

_BUILD_CACHE = {}


def _emit_mm_set(nc, psum_pool, lhsT_tile, rhs_tile, evict, n_mt=DT):
    """One [512,512]-ish matmul set: for each output row-block mt, accumulate
    over DT k-tiles into PSUM and call evict(mt, psum_ap)."""
    for mt in range(n_mt):
        ps = psum_pool.tile([P, BC], f32, tag="ps")
        for kt in range(DT):
            nc.tensor.matmul(
                ps[:],
                lhsT_tile[:, kt, mt * P:(mt + 1) * P],
                rhs_tile[:, kt, :],
                start=(kt == 0),
                stop=(kt == DT - 1),
            )
        evict(mt, ps)


def _build(n_steps: int):
    """Build + compile the Bass module for a given n_steps."""
    n = int(n_steps)
    assert n >= 0
    nc = bacc.Bacc("TRN2", target_bir_lowering=False, debug=False,
                   enable_asserts=False, num_devices=NCORES)

    # f32r-declared DRAM inputs carry raw fp32 bytes; the PE rounds internally
    # (verified bit-identical to an explicit cast) so plain HWDGE DMA works.
    xt_d = nc.dram_tensor("xt", [P, DT * BC], f32, kind="ExternalInput")
    xtr_d = nc.dram_tensor("xtr", [P, DT * BC], f32r, kind="ExternalInput")
    d0_d = nc.dram_tensor("d0", [P, DT * DZ], f32r, kind="ExternalInput")
    t0_d = nc.dram_tensor("t0", [P, DT * DZ], f32r, kind="ExternalInput")
    w1t_d = nc.dram_tensor("w1t", [P, DT * H], f32r, kind="ExternalInput")
    b1t_d = nc.dram_tensor("b1t", [P, HT], f32, kind="ExternalInput")
    w2t_d = nc.dram_tensor("w2t", [P, HT * DY], f32r, kind="ExternalInput")
    b2t_d = nc.dram_tensor("b2t", [DY, 1], f32, kind="ExternalInput")
    ident_d = nc.dram_tensor("ident", [P, P], f32, kind="ExternalInput")
    identr_d = nc.dram_tensor("identr", [P, P], f32r, kind="ExternalInput")
    y_d = nc.dram_tensor("y", [BC, DY], f32, kind="ExternalOutput")

    mult = mybir.AluOpType.mult
    add = mybir.AluOpType.add

    with tile.TileContext(nc) as tc:
        with (
            tc.tile_pool(name="const", bufs=1) as const_pool,
            tc.tile_pool(name="weights", bufs=1) as w_pool,
            tc.tile_pool(name="chain", bufs=2) as chain_pool,
            tc.tile_pool(name="accp", bufs=2) as acc_pool,
            tc.tile_pool(name="acts", bufs=1) as act_pool,
            tc.tile_pool(name="out", bufs=2) as out_pool,
            tc.tile_pool(name="psum", bufs=7, space="PSUM") as psum_pool,
            tc.tile_pool(name="psum_y", bufs=1, space="PSUM") as psum_y_pool,
        ):
            # ---- loads (all fast HWDGE; chain inputs first) ----------------
            def load(dram, shape, tag, dtype=f32r, chunks=1):
                r = w_pool.tile(shape, dtype, tag=tag)
                src = dram.ap().rearrange("p (t b) -> p t b", t=shape[1])
                for ch in range(chunks):
                    lo = shape[1] * ch // chunks
                    hi = shape[1] * (ch + 1) // chunks
                    nc.sync.dma_start(r[:, lo:hi, :], src[:, lo:hi, :])
                return r

            # All loads go through one trigger queue (Sync) in priority order:
            # the DMA rings are FIFO, so t0 — which gates the squaring
            # chain — must be enqueued before the bulk weight loads.
            identr = const_pool.tile([P, P], f32r, tag="identr")
            nc.sync.dma_start(identr[:], identr_d.ap())
            d_cur = w_pool.tile([P, DT, DZ], f32r, tag="d0")
            t_cur = w_pool.tile([P, DT, DZ], f32r, tag="t0")
            d0_src = d0_d.ap().rearrange("p (t b) -> p t b", t=DT)
            t0_src = t0_d.ap().rearrange("p (t b) -> p t b", t=DT)
            for lo, hi in ((0, 2), (2, DT)):
                nc.sync.dma_start(d_cur[:, lo:hi, :], d0_src[:, lo:hi, :])
                nc.sync.dma_start(t_cur[:, lo:hi, :], t0_src[:, lo:hi, :])

            def load(dram, shape, tag, dtype=f32r):
                r = w_pool.tile(shape, dtype, tag=tag)
                nc.sync.dma_start(
                    r[:], dram.ap().rearrange("p (t b) -> p t b", t=shape[1]))
                return r

            xt_r = load(xtr_d, [P, DT, BC], "xtr")
            xt = load(xt_d, [P, DT, BC], "xt", dtype=f32)
            w1t = load(w1t_d, [P, DT, H], "w1t")
            w2t = load(w2t_d, [P, HT, DY], "w2t")

            b1t = const_pool.tile([P, HT], f32, tag="b1t")
            nc.sync.dma_start(b1t[:], b1t_d.ap())
            b2t = const_pool.tile([DY, 1], f32, tag="b2t")
            nc.sync.dma_start(b2t[:], b2t_d.ap())
            ident = const_pool.tile([P, P], f32, tag="ident")
            nc.sync.dma_start(ident[:], ident_d.ap())

            # Brief PE warm-up while waiting for t0: HAM only unthrottles
            # (1.2 -> 2.4 GHz) after ~3.4us of sustained matmul activity, so
            # start the activity window as early as possible. Alternate two
            # PSUM tiles so the dummies pipeline instead of WAW-serializing.
            ps_w0 = psum_y_pool.tile([P, P], f32, tag="psy")
            ps_w1 = psum_pool.tile([P, P], f32, tag="ps")
            for i in range(10):
                nc.tensor.matmul([ps_w0, ps_w1][i % 2][:], identr[:], identr[:],
                                 start=True, stop=True)

            # ---- binary exponentiation on the deviation chain --------------
            acc = xt_r          # zT accumulator, fp32r [P, DT, BC]
            acc_f32 = xt        # exact fp32 twin, used for the fused +acc add

            def apply_T(t_tile, acc_r, acc_exact):
                """acc <- acc + D @ acc   (W^(2^k) application)."""
                new_r = acc_pool.tile([P, DT, BC], f32r, tag="acc")

                def evict(mt, ps):
                    nc.vector.scalar_tensor_tensor(
                        new_r[:, mt, :], acc_exact[:, mt, :], 1.0, ps[:],
                        op0=mult, op1=add)

                _emit_mm_set(nc, psum_pool, t_tile, acc_r, evict)
                return new_r, new_r

            def square_level(d_tile, t_tile, with_d):
                """One chain level: T' = 2T + T@T (and D' = 2D + D@D when
                still needed). T and D sets interleave per output tile mt so
                that evictions for k-tile kt land early — the next level's
                MM(mt, kt) only needs the kt-th evictions, so levels overlap
                with no PE bubble."""
                t_new = chain_pool.tile([P, DT, DZ], f32r, tag="T")
                if with_d:
                    d_new = chain_pool.tile([P, DT, DZ], f32r, tag="D")
                else:
                    d_new = None
                for mt in range(DT):
                    ps_t = psum_pool.tile([P, BC], f32, tag="ps")
                    for kt in range(DT):
                        nc.tensor.matmul(
                            ps_t[:], d_tile[:, kt, mt * P:(mt + 1) * P],
                            t_tile[:, kt, :], start=(kt == 0), stop=(kt == DT - 1))
                    nc.vector.scalar_tensor_tensor(
                        t_new[:, mt, :], t_tile[:, mt, :], 2.0, ps_t[:],
                        op0=mult, op1=add)
                    if with_d:
                        ps_d = psum_pool.tile([P, BC], f32, tag="ps")
                        for kt in range(DT):
                            nc.tensor.matmul(
                                ps_d[:], t_tile[:, kt, mt * P:(mt + 1) * P],
                                d_tile[:, kt, :], start=(kt == 0), stop=(kt == DT - 1))
                        nc.vector.scalar_tensor_tensor(
                            d_new[:, mt, :], d_tile[:, mt, :], 2.0, ps_d[:],
                            op0=mult, op1=add)
                return t_new, d_new

            if n == 1:
                acc, acc_f32 = apply_T(t_cur, acc, acc_f32)
            elif n > 1:
                # Binary exponentiation; the top bit is applied as a *fused
                # double application* of T_{mb-1}:
                #   u = T^t @ acc ; z = acc + 2u + T^t @ u
                # which skips the last chain level entirely (T_mb and D_{mb-1}
                # sets are never built): 32 fewer matmuls at ~no accuracy cost.
                mb = n.bit_length() - 1
                for k in range(0, mb):
                    if (n >> k) & 1:
                        acc, acc_f32 = apply_T(t_cur, acc, acc_f32)
                    if k < mb - 1:
                        t_cur, d_cur = square_level(d_cur, t_cur,
                                                    with_d=(k + 1 < mb - 1))
                u = acc_pool.tile([P, DT, BC], f32r, tag="uacc")
                pre = acc_pool.tile([P, DT, BC], f32, tag="upre")
                for mt in range(DT):
                    ps = psum_pool.tile([P, BC], f32, tag="ps")
                    for kt in range(DT):
                        nc.tensor.matmul(
                            ps[:], t_cur[:, kt, mt * P:(mt + 1) * P],
                            acc[:, kt, :], start=(kt == 0), stop=(kt == DT - 1))
                    nc.scalar.activation(
                        u[:, mt, :], ps[:], mybir.ActivationFunctionType.Copy)
                    # pre = acc + 2u, off the critical path (overlaps the
                    # second matmul set); the final evict is then a single op.
                    nc.vector.scalar_tensor_tensor(
                        pre[:, mt, :], u[:, mt, :], 2.0, acc_f32[:, mt, :],
                        op0=mult, op1=add)
                znew = acc_pool.tile([P, DT, BC], f32r, tag="acc")
                for mt in range(DT):
                    ps = psum_pool.tile([P, BC], f32, tag="ps")
                    for kt in range(DT):
                        nc.tensor.matmul(
                            ps[:], t_cur[:, kt, mt * P:(mt + 1) * P],
                            u[:, kt, :], start=(kt == 0), stop=(kt == DT - 1))
                    nc.vector.scalar_tensor_tensor(
                        znew[:, mt, :], pre[:, mt, :], 1.0, ps[:],
                        op0=mult, op1=add)
                acc = znew

            zt = acc  # fp32r [P, DT, BC]

            # ---- MLP: hT = relu(W1 @ z + b1); yT = W2 @ h + b2 -------------
            # Layer-2 accumulation MMs interleave with layer-1 so the tail
            # after the last h-tile is just one MM + bias + transpose.
            ht = act_pool.tile([P, HT, BC], f32r, tag="ht")
            ps_y = psum_y_pool.tile([DY, BC], f32, tag="psy")
            for mt in range(HT):
                ps = psum_pool.tile([P, BC], f32, tag="ps")
                for kt in range(DT):
                    nc.tensor.matmul(
                        ps[:], w1t[:, kt, mt * P:(mt + 1) * P], zt[:, kt, :],
                        start=(kt == 0), stop=(kt == DT - 1))
                nc.scalar.activation(
                    ht[:, mt, :], ps[:], mybir.ActivationFunctionType.Relu,
                    bias=b1t[:, mt:mt + 1])
                nc.tensor.matmul(ps_y[:], w2t[:, mt, :], ht[:, mt, :],
                                 start=(mt == 0), stop=(mt == HT - 1))
            ytb = out_pool.tile([DY, BC], f32, tag="ytb")
            nc.scalar.activation(ytb[:], ps_y[:],
                                 mybir.ActivationFunctionType.Identity,
                                 bias=b2t[:])

            # ---- transpose yT -> y and store -------------------------------
            y_sb = out_pool.tile([P, BC // P, DY], f32, tag="ysb")
            for bt in range(BC // P):
                ps_t = psum_y_pool.tile([P, DY], f32, tag="psy")
                nc.tensor.transpose(
                    ps_t[:], ytb[:, bt * P:(bt + 1) * P], ident[:DY, :DY])
                nc.vector.tensor_copy(y_sb[:, bt, :], ps_t[:])
            nc.sync.dma_start(
                y_d.ap().rearrange("(bt p) j -> p bt j", p=P), y_sb[:])

    nc.compile()
    return nc


def _tiles_pk(m: np.ndarray) -> np.ndarray:
    """[nt*128, C] -> [128, nt*C] partition-tiled layout (row r = kt*128+p)."""
    nt = m.shape[0] // P
    return np.ascontiguousarray(m.reshape(nt, P, -1).swapaxes(0, 1)).reshape(P, -1)


def kernel(x, A, W1, b1, W2, b2, n_steps) -> np.ndarray:
    x = np.asarray(x, dtype=np.float32)
    A = np.asarray(A, dtype=np.float32)
    W1 = np.asarray(W1, dtype=np.float32)
    b1 = np.asarray(b1, dtype=np.float32)
    W2 = np.asarray(W2, dtype=np.float32)
    b2 = np.asarray(b2, dtype=np.float32)
    n = int(np.asarray(n_steps))

    if n not in _BUILD_CACHE:
        _BUILD_CACHE[n] = _build(n)
    nc = _BUILD_CACHE[n]

    dt = np.float32(1.0 / n) if n > 0 else np.float32(0.0)
    d0 = _tiles_pk(np.ascontiguousarray(dt * A, dtype=np.float32))
    t0 = _tiles_pk(np.ascontiguousarray(dt * A.T, dtype=np.float32))
    w1t = _tiles_pk(np.ascontiguousarray(W1.T))           # [512, 2048]
    w2t = _tiles_pk(np.ascontiguousarray(W2.T))           # [2048, 10]
    b1t = np.ascontiguousarray(b1.reshape(HT, P).T)       # [128, 16]
    b2t = np.ascontiguousarray(b2.reshape(DY, 1))
    ident = np.eye(P, dtype=np.float32)

    in_maps = []
    for c in range(NCORES):
        xs = x[c * BC:(c + 1) * BC, :]                    # [512, 512]
        xt = _tiles_pk(np.ascontiguousarray(xs.T))        # [128, 4*512]
        in_maps.append({
            "xt": xt, "xtr": xt, "d0": d0, "t0": t0, "w1t": w1t, "b1t": b1t,
            "w2t": w2t, "b2t": b2t, "ident": ident, "identr": ident,
        })

    trace = bool(os.environ.get("BASS_KERNEL_TRACE"))
    kwargs = {}
    if trace:
        kwargs = {"trace": True, "trace_cores": [0]}
    res = run_bass_kernel_spmd(nc, in_maps, list(range(NCORES)), **kwargs)
    if trace and res.exec_time_ns is not None:
        print(f"HW exec time: {res.exec_time_ns} ns")

    y = np.concatenate([res.results[c]["y"] for c in range(NCORES)], axis=0)
    return y.astype(np.float32)


# revision 34
# speedup vs baseline: 1.0242x; 1.0044x over previous
"""Trainium2 Bass kernel for MatrixOdeGradientDescentModel.

Reference computation (B=4096, DZ=512, H=2048, DY=10, n_steps=64):
    z = x; repeat n_steps: z += dt * z @ A.T          (dt = 1/n_steps)
    y = relu(z @ W1.T + b1) @ W2.T + b2

Algebraic rewrite: the Euler loop is linear, so
    z_final = x @ (M^T)^n  with  M = I + dt*A.
We compute W := M^T = I + dt*A^T by repeated squaring on the *deviation*
D_k := W^(2^k) - I (avoids precision loss from the identity's magnitude):
    D_{k+1} = 2*D_k + D_k @ D_k
maintaining the pair (D_k, T_k=D_k^T) so no on-device transposes are needed:
    D@D = matmul(lhsT=T, rhs=D),   (D@D)^T = matmul(lhsT=D, rhs=T)
then zT = xT + D_chain applied to xT per set bit of n (binary exponentiation).

Sharding: data-parallel over batch. Each of the 8 cores gets 512 rows of x;
A/W1/W2 replicated; no cross-core communication.

Matmuls run in float32r (TF32-like, 4x faster than fp32 on the PE) with fp32
PSUM accumulation; the error-compensated deviation chain keeps the end-to-end
relative error at the ~1e-4 level.
"""

import os

import numpy as np

import concourse.bacc as bacc
import concourse.mybir as mybir
import concourse.tile as tile
from concourse.bass_utils import run_bass_kernel_spmd

P = 128
B, DZ, H, DY = 4096, 512, 2048, 10
NCORES = 8
BC = B // NCORES          # 512 rows per core
DT = DZ // P              # 4 k-tiles over DZ
HT = H // P               # 16 m-tiles over H

f32 = mybir.dt.float32
f32r = mybir.dt.float32r

_BUILD_CACHE = {}


def _emit_mm_set(nc, psum_pool, lhsT_tile, rhs_tile, evict, n_mt=DT):
    """One [512,512]-ish matmul set: for each output row-block mt, accumulate
    over DT k-tiles into PSUM and call evict(mt, psum_ap)."""
    for mt in range(n_mt):
        ps = psum_pool.tile([P, BC], f32, tag="ps")
        for kt in range(DT):
            nc.tensor.matmul(
                ps[:],
                lhsT_tile[:, kt, mt * P:(mt + 1) * P],
                rhs_tile[:, kt, :],
                start=(kt == 0),
                stop=(kt == DT - 1),
            )
        evict(mt, ps)


def _build(n_steps: int):
    """Build + compile the Bass module for a given n_steps."""
    n = int(n_steps)
    assert n >= 0
    nc = bacc.Bacc("TRN2", target_bir_lowering=False, debug=False,
                   enable_asserts=False, num_devices=NCORES)

    # f32r-declared DRAM inputs carry raw fp32 bytes; the PE rounds internally
    # (verified bit-identical to an explicit cast) so plain HWDGE DMA works.
    xt_d = nc.dram_tensor("xt", [P, DT * BC], f32, kind="ExternalInput")
    xtr_d = nc.dram_tensor("xtr", [P, DT * BC], f32r, kind="ExternalInput")
    d0_d = nc.dram_tensor("d0", [P, DT * DZ], f32r, kind="ExternalInput")
    t0_d = nc.dram_tensor("t0", [P, DT * DZ], f32r, kind="ExternalInput")
    w1t_d = nc.dram_tensor("w1t", [P, DT * H], f32r, kind="ExternalInput")
    b1t_d = nc.dram_tensor("b1t", [P, HT], f32, kind="ExternalInput")
    w2t_d = nc.dram_tensor("w2t", [P, HT * DY], f32r, kind="ExternalInput")
    b2t_d = nc.dram_tensor("b2t", [DY, 1], f32, kind="ExternalInput")
    ident_d = nc.dram_tensor("ident", [P, P], f32, kind="ExternalInput")
    identr_d = nc.dram_tensor("identr", [P, P], f32r, kind="ExternalInput")
    y_d = nc.dram_tensor("y", [BC, DY], f32, kind="ExternalOutput")

    mult = mybir.AluOpType.mult
    add = mybir.AluOpType.add

    with tile.TileContext(nc) as tc:
        with (
            tc.tile_pool(name="const", bufs=1) as const_pool,
            tc.tile_pool(name="weights", bufs=1) as w_pool,
            tc.tile_pool(name="chain", bufs=2) as chain_pool,
            tc.tile_pool(name="accp", bufs=2) as acc_pool,
            tc.tile_pool(name="acts", bufs=1) as act_pool,
            tc.tile_pool(name="out", bufs=2) as out_pool,
            tc.tile_pool(name="psum", bufs=7, space="PSUM") as psum_pool,
            tc.tile_pool(name="psum_y", bufs=1, space="PSUM") as psum_y_pool,
        ):
            # ---- loads (all fast HWDGE; chain inputs first) ----------------
            def load(dram, shape, tag, dtype=f32r, chunks=1):
                r = w_pool.tile(shape, dtype, tag=tag)
                src = dram.ap().rearrange("p (t b) -> p t b", t=shape[1])
                for ch in range(chunks):
                    lo = shape[1] * ch // chunks
                    hi = shape[1] * (ch + 1) // chunks
                    nc.sync.dma_start(r[:, lo:hi, :], src[:, lo:hi, :])
                return r

            # All loads go through one trigger queue (Sync) in priority order:
            # the DMA rings are FIFO, so t0 — which gates the squaring
            # chain — must be enqueued before the bulk weight loads.
            identr = const_pool.tile([P, P], f32r, tag="identr")
            nc.sync.dma_start(identr[:], identr_d.ap())
            d_cur = w_pool.tile([P, DT, DZ], f32r, tag="d0")
            t_cur = w_pool.tile([P, DT, DZ], f32r, tag="t0")
            d0_src = d0_d.ap().rearrange("p (t b) -> p t b", t=DT)
            t0_src = t0_d.ap().rearrange("p (t b) -> p t b", t=DT)
            for lo, hi in ((0, 2), (2, DT)):
                nc.sync.dma_start(d_cur[:, lo:hi, :], d0_src[:, lo:hi, :])
                nc.sync.dma_start(t_cur[:, lo:hi, :], t0_src[:, lo:hi, :])

            def load(dram, shape, tag, dtype=f32r):
                r = w_pool.tile(shape, dtype, tag=tag)
                nc.sync.dma_start(
                    r[:], dram.ap().rearrange("p (t b) -> p t b", t=shape[1]))
                return r

            xt_r = load(xtr_d, [P, DT, BC], "xtr")
            xt = load(xt_d, [P, DT, BC], "xt", dtype=f32)
            w1t = load(w1t_d, [P, DT, H], "w1t")
            w2t = load(w2t_d, [P, HT, DY], "w2t")

            b1t = const_pool.tile([P, HT], f32, tag="b1t")
            nc.sync.dma_start(b1t[:], b1t_d.ap())
            b2t = const_pool.tile([DY, 1], f32, tag="b2t")
            nc.sync.dma_start(b2t[:], b2t_d.ap())
            ident = const_pool.tile([P, P], f32, tag="ident")
            nc.sync.dma_start(ident[:], ident_d.ap())

            # Brief PE warm-up while waiting for t0: HAM only unthrottles
            # (1.2 -> 2.4 GHz) after ~3.4us of sustained matmul activity, so
            # start the activity window as early as possible. Alternate two
            # PSUM tiles so the dummies pipeline instead of WAW-serializing.
            ps_w0 = psum_y_pool.tile([P, P], f32, tag="psy")
            ps_w1 = psum_pool.tile([P, P], f32, tag="ps")
            for i in range(26):
                nc.tensor.matmul([ps_w0, ps_w1][i % 2][:], identr[:], identr[:],
                                 start=True, stop=True)

            # ---- binary exponentiation on the deviation chain --------------
            acc = xt_r          # zT accumulator, fp32r [P, DT, BC]
            acc_f32 = xt        # exact fp32 twin, used for the fused +acc add

            def apply_T(t_tile, acc_r, acc_exact):
                """acc <- acc + D @ acc   (W^(2^k) application)."""
                new_r = acc_pool.tile([P, DT, BC], f32r, tag="acc")

                def evict(mt, ps):
                    nc.vector.scalar_tensor_tensor(
                        new_r[:, mt, :], acc_exact[:, mt, :], 1.0, ps[:],
                        op0=mult, op1=add)

                _emit_mm_set(nc, psum_pool, t_tile, acc_r, evict)
                return new_r, new_r

            def square_level(d_tile, t_tile, with_d):
                """One chain level: T' = 2T + T@T (and D' = 2D + D@D when
                still needed). T and D sets interleave per output tile mt so
                that evictions for k-tile kt land early — the next level's
                MM(mt, kt) only needs the kt-th evictions, so levels overlap
                with no PE bubble."""
                t_new = chain_pool.tile([P, DT, DZ], f32r, tag="T")
                if with_d:
                    d_new = chain_pool.tile([P, DT, DZ], f32r, tag="D")
                else:
                    d_new = None
                for mt in range(DT):
                    ps_t = psum_pool.tile([P, BC], f32, tag="ps")
                    for kt in range(DT):
                        nc.tensor.matmul(
                            ps_t[:], d_tile[:, kt, mt * P:(mt + 1) * P],
                            t_tile[:, kt, :], start=(kt == 0), stop=(kt == DT - 1))
                    nc.vector.scalar_tensor_tensor(
                        t_new[:, mt, :], t_tile[:, mt, :], 2.0, ps_t[:],
                        op0=mult, op1=add)
                    if with_d:
                        ps_d = psum_pool.tile([P, BC], f32, tag="ps")
                        for kt in range(DT):
                            nc.tensor.matmul(
                                ps_d[:], t_tile[:, kt, mt * P:(mt + 1) * P],
                                d_tile[:, kt, :], start=(kt == 0), stop=(kt == DT - 1))
                        nc.vector.scalar_tensor_tensor(
                            d_new[:, mt, :], d_tile[:, mt, :], 2.0, ps_d[:],
                            op0=mult, op1=add)
                return t_new, d_new

            if n == 1:
                acc, acc_f32 = apply_T(t_cur, acc, acc_f32)
            elif n > 1:
                # Binary exponentiation; the top bit is applied as a *fused
                # double application* of T_{mb-1}:
                #   u = T^t @ acc ; z = acc + 2u + T^t @ u
                # which skips the last chain level entirely (T_mb and D_{mb-1}
                # sets are never built): 32 fewer matmuls at ~no accuracy cost.
                mb = n.bit_length() - 1
                for k in range(0, mb):
                    if (n >> k) & 1:
                        acc, acc_f32 = apply_T(t_cur, acc, acc_f32)
                    if k < mb - 1:
                        t_cur, d_cur = square_level(d_cur, t_cur,
                                                    with_d=(k + 1 < mb - 1))
                u = acc_pool.tile([P, DT, BC], f32r, tag="uacc")
                pre = acc_pool.tile([P, DT, BC], f32, tag="upre")
                for mt in range(DT):
                    ps = psum_pool.tile([P, BC], f32, tag="ps")
                    for kt in range(DT):
                        nc.tensor.matmul(
                            ps[:], t_cur[:, kt, mt * P:(mt + 1) * P],
                            acc[:, kt, :], start=(kt == 0), stop=(kt == DT - 1))
                    nc.scalar.activation(
                        u[:, mt, :], ps[:], mybir.ActivationFunctionType.Copy)
                    # pre = acc + 2u, off the critical path (overlaps the
                    # second matmul set); the final evict is then a single op.
                    nc.vector.scalar_tensor_tensor(
                        pre[:, mt, :], u[:, mt, :], 2.0, acc_f32[:, mt, :],
                        op0=mult, op1=add)
                znew = acc_pool.tile([P, DT, BC], f32r, tag="acc")
                for mt in range(DT):
                    ps = psum_pool.tile([P, BC], f32, tag="ps")
                    for kt in range(DT):
                        nc.tensor.matmul(
                            ps[:], t_cur[:, kt, mt * P:(mt + 1) * P],
                            u[:, kt, :], start=(kt == 0), stop=(kt == DT - 1))
                    nc.vector.scalar_tensor_tensor(
                        znew[:, mt, :], pre[:, mt, :], 1.0, ps[:],
                        op0=mult, op1=add)
                acc = znew

            zt = acc  # fp32r [P, DT, BC]

            # ---- MLP: hT = relu(W1 @ z + b1); yT = W2 @ h + b2 -------------
            # Layer-2 accumulation MMs interleave with layer-1 so the tail
            # after the last h-tile is just one MM + bias + transpose.
            ht = act_pool.tile([P, HT, BC], f32r, tag="ht")
            ps_y = psum_y_pool.tile([DY, BC], f32, tag="psy")
            for mt in range(HT):
                ps = psum_pool.tile([P, BC], f32, tag="ps")
                for kt in range(DT):
                    nc.tensor.matmul(
                        ps[:], w1t[:, kt, mt * P:(mt + 1) * P], zt[:, kt, :],
                        start=(kt == 0), stop=(kt == DT - 1))
                nc.scalar.activation(
                    ht[:, mt, :], ps[:], mybir.ActivationFunctionType.Relu,
                    bias=b1t[:, mt:mt + 1])
                nc.tensor.matmul(ps_y[:], w2t[:, mt, :], ht[:, mt, :],
                                 start=(mt == 0), stop=(mt == HT - 1))
            ytb = out_pool.tile([DY, BC], f32, tag="ytb")
            nc.scalar.activation(ytb[:], ps_y[:],
                                 mybir.ActivationFunctionType.Identity,
                                 bias=b2t[:])

            # ---- transpose yT -> y and store -------------------------------
            y_sb = out_pool.tile([P, BC // P, DY], f32, tag="ysb")
            for bt in range(BC // P):
                ps_t = psum_y_pool.tile([P, DY], f32, tag="psy")
                nc.tensor.transpose(
                    ps_t[:], ytb[:, bt * P:(bt + 1) * P], ident[:DY, :DY])
                nc.vector.tensor_copy(y_sb[:, bt, :], ps_t[:])
            nc.sync.dma_start(
                y_d.ap().rearrange("(bt p) j -> p bt j", p=P), y_sb[:])

    nc.compile()
    return nc


def _tiles_pk(m: np.ndarray) -> np.ndarray:
    """[nt*128, C] -> [128, nt*C] partition-tiled layout (row r = kt*128+p)."""
    nt = m.shape[0] // P
    return np.ascontiguousarray(m.reshape(nt, P, -1).swapaxes(0, 1)).reshape(P, -1)


def kernel(x, A, W1, b1, W2, b2, n_steps) -> np.ndarray:
    x = np.asarray(x, dtype=np.float32)
    A = np.asarray(A, dtype=np.float32)
    W1 = np.asarray(W1, dtype=np.float32)
    b1 = np.asarray(b1, dtype=np.float32)
    W2 = np.asarray(W2, dtype=np.float32)
    b2 = np.asarray(b2, dtype=np.float32)
    n = int(np.asarray(n_steps))

    if n not in _BUILD_CACHE:
        _BUILD_CACHE[n] = _build(n)
    nc = _BUILD_CACHE[n]

    dt = np.float32(1.0 / n) if n > 0 else np.float32(0.0)
    d0 = _tiles_pk(np.ascontiguousarray(dt * A, dtype=np.float32))
    t0 = _tiles_pk(np.ascontiguousarray(dt * A.T, dtype=np.float32))
    w1t = _tiles_pk(np.ascontiguousarray(W1.T))           # [512, 2048]
    w2t = _tiles_pk(np.ascontiguousarray(W2.T))           # [2048, 10]
    b1t = np.ascontiguousarray(b1.reshape(HT, P).T)       # [128, 16]
    b2t = np.ascontiguousarray(b2.reshape(DY, 1))
    ident = np.eye(P, dtype=np.float32)

    in_maps = []
    for c in range(NCORES):
        xs = x[c * BC:(c + 1) * BC, :]                    # [512, 512]
        xt = _tiles_pk(np.ascontiguousarray(xs.T))        # [128, 4*512]
        in_maps.append({
            "xt": xt, "xtr": xt, "d0": d0, "t0": t0, "w1t": w1t, "b1t": b1t,
            "w2t": w2t, "b2t": b2t, "ident": ident, "identr": ident,
        })

    trace = bool(os.environ.get("BASS_KERNEL_TRACE"))
    kwargs = {}
    if trace:
        kwargs = {"trace": True, "trace_cores": [0]}
    res = run_bass_kernel_spmd(nc, in_maps, list(range(NCORES)), **kwargs)
    if trace and res.exec_time_ns is not None:
        print(f"HW exec time: {res.exec_time_ns} ns")

    y = np.concatenate([res.results[c]["y"] for c in range(NCORES)], axis=0)
    return y.astype(np.float32)


# revision 37
# speedup vs baseline: 1.0536x; 1.0288x over previous
"""Trainium2 Bass kernel for MatrixOdeGradientDescentModel.

Reference computation (B=4096, DZ=512, H=2048, DY=10, n_steps=64):
    z = x; repeat n_steps: z += dt * z @ A.T          (dt = 1/n_steps)
    y = relu(z @ W1.T + b1) @ W2.T + b2

Algebraic rewrite: the Euler loop is linear, so
    z_final = x @ P^T with P^T = (W)^n,  W = I + dt*A^T  (T0 := dt*A^T).
(W)^n = sum_k C(n,k) T0^k. Since ||T0|| = ||A||/n (~0.014 here), the series
truncated at degree 9 has ~1e-7 relative tail for any n (C(n,k)/n^k <= 1/k!),
so we evaluate it Paterson-Stockmeyer style with X = T0^2:
    P_dev = c1*T0 + X*(B1 + X*(B2 + X*(B3 + X*B4)))     [P = I + P_dev]
where B_j = c_{2j}*I + c_{2j+1}*T0 are HOST-computed (O(n^2) elementwise) and
folded into the PSUM evictions, X is applied via its transpose D0^2
(D0 := dt*A, so D0^2 = (T0^2)^T is one matmul set away — no on-device
transposes). Then zT = xT + P_dev-apply(xT), and the MLP.

Sharding: data-parallel over batch. Each of the 8 cores gets 512 rows of x;
A/W1/W2 replicated; no cross-core communication.

Matmuls run in float32r (TF32-like, 4x faster than fp32 on the PE) with fp32
PSUM accumulation; the identity-free deviation formulation keeps the
end-to-end relative error at the ~2e-4 level.
"""

import os
from math import comb

import numpy as np

import concourse.bacc as bacc
import concourse.mybir as mybir
import concourse.tile as tile
from concourse.bass_utils import run_bass_kernel_spmd

P = 128
B, DZ, H, DY = 4096, 512, 2048, 10
NCORES = 8
BC = B // NCORES          # 512 rows per core
DT = DZ // P              # 4 k-tiles over DZ
HT = H // P               # 16 m-tiles over H

f32 = mybir.dt.float32
f32r = mybir.dt.float32r

_BUILD_CACHE = {}


def _emit_mm_set(nc, psum_pool, lhsT_tile, rhs_tile, evict, n_mt=DT):
    """One [512,512]-ish matmul set: for each output row-block mt, accumulate
    over DT k-tiles into PSUM and call evict(mt, psum_ap)."""
    for mt in range(n_mt):
        ps = psum_pool.tile([P, BC], f32, tag="ps")
        for kt in range(DT):
            nc.tensor.matmul(
                ps[:],
                lhsT_tile[:, kt, mt * P:(mt + 1) * P],
                rhs_tile[:, kt, :],
                start=(kt == 0),
                stop=(kt == DT - 1),
            )
        evict(mt, ps)


def _build(n_steps: int):
    """Build + compile the Bass module for a given n_steps."""
    n = int(n_steps)
    assert n >= 0
    nc = bacc.Bacc("TRN2", target_bir_lowering=False, debug=False,
                   enable_asserts=False, num_devices=NCORES)

    # f32r-declared DRAM inputs carry raw fp32 bytes; the PE rounds internally
    # (verified bit-identical to an explicit cast) so plain HWDGE DMA works.
    xt_d = nc.dram_tensor("xt", [P, DT * BC], f32, kind="ExternalInput")
    xtr_d = nc.dram_tensor("xtr", [P, DT * BC], f32r, kind="ExternalInput")
    d0_d = nc.dram_tensor("d0", [P, DT * DZ], f32r, kind="ExternalInput")
    t0_d = nc.dram_tensor("t0", [P, DT * DZ], f32r, kind="ExternalInput")
    y4_d = nc.dram_tensor("y4", [P, DT * DZ], f32r, kind="ExternalInput")
    bs_d = nc.dram_tensor("bs", [P, 3 * DT * DZ], f32, kind="ExternalInput")
    w1t_d = nc.dram_tensor("w1t", [P, DT * H], f32r, kind="ExternalInput")
    b1t_d = nc.dram_tensor("b1t", [P, HT], f32, kind="ExternalInput")
    w2t_d = nc.dram_tensor("w2t", [P, HT * DY], f32r, kind="ExternalInput")
    b2t_d = nc.dram_tensor("b2t", [DY, 1], f32, kind="ExternalInput")
    ident_d = nc.dram_tensor("ident", [P, P], f32, kind="ExternalInput")
    identr_d = nc.dram_tensor("identr", [P, P], f32r, kind="ExternalInput")
    y_d = nc.dram_tensor("y", [BC, DY], f32, kind="ExternalOutput")

    mult = mybir.AluOpType.mult
    add = mybir.AluOpType.add

    with tile.TileContext(nc) as tc:
        with (
            tc.tile_pool(name="const", bufs=1) as const_pool,
            tc.tile_pool(name="weights", bufs=1) as w_pool,
            tc.tile_pool(name="horner", bufs=2) as horner_pool,
            tc.tile_pool(name="accp", bufs=2) as acc_pool,
            tc.tile_pool(name="acts", bufs=1) as act_pool,
            tc.tile_pool(name="out", bufs=2) as out_pool,
            tc.tile_pool(name="psum", bufs=7, space="PSUM") as psum_pool,
            tc.tile_pool(name="psum_y", bufs=1, space="PSUM") as psum_y_pool,
        ):
            # ---- loads (all fast HWDGE; chain inputs first) ----------------
            def load(dram, shape, tag, dtype=f32r, chunks=1):
                r = w_pool.tile(shape, dtype, tag=tag)
                src = dram.ap().rearrange("p (t b) -> p t b", t=shape[1])
                for ch in range(chunks):
                    lo = shape[1] * ch // chunks
                    hi = shape[1] * (ch + 1) // chunks
                    nc.sync.dma_start(r[:, lo:hi, :], src[:, lo:hi, :])
                return r

            # All loads go through one trigger queue (Sync) in priority order:
            # the DMA rings are FIFO, so t0 — which gates the squaring
            # chain — must be enqueued before the bulk weight loads.
            identr = const_pool.tile([P, P], f32r, tag="identr")
            nc.sync.dma_start(identr[:], identr_d.ap())
            d_cur = w_pool.tile([P, DT, DZ], f32r, tag="d0")
            t_cur = w_pool.tile([P, DT, DZ], f32r, tag="t0")
            d0_src = d0_d.ap().rearrange("p (t b) -> p t b", t=DT)
            t0_src = t0_d.ap().rearrange("p (t b) -> p t b", t=DT)
            for lo, hi in ((0, 2), (2, DT)):
                nc.sync.dma_start(d_cur[:, lo:hi, :], d0_src[:, lo:hi, :])
                nc.sync.dma_start(t_cur[:, lo:hi, :], t0_src[:, lo:hi, :])

            def load(dram, shape, tag, dtype=f32r):
                r = w_pool.tile(shape, dtype, tag=tag)
                nc.sync.dma_start(
                    r[:], dram.ap().rearrange("p (t b) -> p t b", t=shape[1]))
                return r

            y4t = load(y4_d, [P, DT, DZ], "y4")
            bs = load(bs_d, [P, 3 * DT, DZ], "bs", dtype=f32)
            xt_r = load(xtr_d, [P, DT, BC], "xtr")
            xt = load(xt_d, [P, DT, BC], "xt", dtype=f32)
            w1t = load(w1t_d, [P, DT, H], "w1t")
            w2t = load(w2t_d, [P, HT, DY], "w2t")

            b1t = const_pool.tile([P, HT], f32, tag="b1t")
            nc.sync.dma_start(b1t[:], b1t_d.ap())
            b2t = const_pool.tile([DY, 1], f32, tag="b2t")
            nc.sync.dma_start(b2t[:], b2t_d.ap())
            ident = const_pool.tile([P, P], f32, tag="ident")
            nc.sync.dma_start(ident[:], ident_d.ap())

            # Brief PE warm-up while waiting for t0: HAM only unthrottles
            # (1.2 -> 2.4 GHz) after ~3.4us of sustained matmul activity, so
            # start the activity window as early as possible. Alternate two
            # PSUM tiles so the dummies pipeline instead of WAW-serializing.
            ps_w0 = psum_y_pool.tile([P, P], f32, tag="psy")
            ps_w1 = psum_pool.tile([P, P], f32, tag="ps")
            for i in range(26):
                nc.tensor.matmul([ps_w0, ps_w1][i % 2][:], identr[:], identr[:],
                                 start=True, stop=True)

            # ---- binary exponentiation on the deviation chain --------------
            acc = xt_r          # zT accumulator, fp32r [P, DT, BC]
            acc_f32 = xt        # exact fp32 twin, used for the fused +acc add

            def apply_T(t_tile, acc_r, acc_exact):
                """acc <- acc + D @ acc   (W^(2^k) application)."""
                new_r = acc_pool.tile([P, DT, BC], f32r, tag="acc")

                def evict(mt, ps):
                    nc.vector.scalar_tensor_tensor(
                        new_r[:, mt, :], acc_exact[:, mt, :], 1.0, ps[:],
                        op0=mult, op1=add)

                _emit_mm_set(nc, psum_pool, t_tile, acc_r, evict)
                return new_r, new_r

            if n == 0:
                zt = xt_r
            elif n == 1:
                zt, _ = apply_T(t_cur, acc, acc_f32)
            else:
                # ---- Paterson-Stockmeyer, X = T0^2, degree 9 --------------
                # Stage A: X as its transpose D0^2 (the lhsT for X-products).
                x2 = w_pool.tile([P, DT, DZ], f32r, tag="x2")

                def evict_x2(mt, ps):
                    nc.scalar.activation(
                        x2[:, mt, :], ps[:], mybir.ActivationFunctionType.Copy)

                _emit_mm_set(nc, psum_pool, t_cur, d_cur, evict_x2)

                # Horner levels: Y_j = B_j + X @ Y_{j+1}, B_j host-provided.
                y_r = y4t
                for j in (3, 2, 1):
                    ynew = horner_pool.tile([P, DT, DZ], f32r, tag="ylev")

                    def evict_y(mt, ps, ynew=ynew, j=j):
                        nc.vector.scalar_tensor_tensor(
                            ynew[:, mt, :], bs[:, (j - 1) * DT + mt, :], 1.0,
                            ps[:], op0=mult, op1=add)

                    _emit_mm_set(nc, psum_pool, x2, y_r, evict_y)
                    y_r = ynew

                # P_dev = c1*T0 + X @ Y1  (c1 = n)
                pd = w_pool.tile([P, DT, DZ], f32r, tag="pd")
                c1 = float(n)

                def evict_pd(mt, ps):
                    nc.vector.scalar_tensor_tensor(
                        pd[:, mt, :], t_cur[:, mt, :], c1, ps[:],
                        op0=mult, op1=add)

                _emit_mm_set(nc, psum_pool, x2, y_r, evict_pd)

                # zT = xT + P_dev-rows @ xT
                zt, _ = apply_T(pd, acc, acc_f32)



            # ---- MLP: hT = relu(W1 @ z + b1); yT = W2 @ h + b2 -------------
            # Layer-2 accumulation MMs interleave with layer-1 so the tail
            # after the last h-tile is just one MM + bias + transpose.
            ht = act_pool.tile([P, HT, BC], f32r, tag="ht")
            ps_y = psum_y_pool.tile([DY, BC], f32, tag="psy")
            for mt in range(HT):
                ps = psum_pool.tile([P, BC], f32, tag="ps")
                for kt in range(DT):
                    nc.tensor.matmul(
                        ps[:], w1t[:, kt, mt * P:(mt + 1) * P], zt[:, kt, :],
                        start=(kt == 0), stop=(kt == DT - 1))
                nc.scalar.activation(
                    ht[:, mt, :], ps[:], mybir.ActivationFunctionType.Relu,
                    bias=b1t[:, mt:mt + 1])
                nc.tensor.matmul(ps_y[:], w2t[:, mt, :], ht[:, mt, :],
                                 start=(mt == 0), stop=(mt == HT - 1))
            ytb = out_pool.tile([DY, BC], f32, tag="ytb")
            nc.scalar.activation(ytb[:], ps_y[:],
                                 mybir.ActivationFunctionType.Identity,
                                 bias=b2t[:])

            # ---- transpose yT -> y and store -------------------------------
            y_sb = out_pool.tile([P, BC // P, DY], f32, tag="ysb")
            for bt in range(BC // P):
                ps_t = psum_y_pool.tile([P, DY], f32, tag="psy")
                nc.tensor.transpose(
                    ps_t[:], ytb[:, bt * P:(bt + 1) * P], ident[:DY, :DY])
                nc.vector.tensor_copy(y_sb[:, bt, :], ps_t[:])
            nc.sync.dma_start(
                y_d.ap().rearrange("(bt p) j -> p bt j", p=P), y_sb[:])

    nc.compile()
    return nc


def _tiles_pk(m: np.ndarray) -> np.ndarray:
    """[nt*128, C] -> [128, nt*C] partition-tiled layout (row r = kt*128+p)."""
    nt = m.shape[0] // P
    return np.ascontiguousarray(m.reshape(nt, P, -1).swapaxes(0, 1)).reshape(P, -1)


def kernel(x, A, W1, b1, W2, b2, n_steps) -> np.ndarray:
    x = np.asarray(x, dtype=np.float32)
    A = np.asarray(A, dtype=np.float32)
    W1 = np.asarray(W1, dtype=np.float32)
    b1 = np.asarray(b1, dtype=np.float32)
    W2 = np.asarray(W2, dtype=np.float32)
    b2 = np.asarray(b2, dtype=np.float32)
    n = int(np.asarray(n_steps))

    if n not in _BUILD_CACHE:
        _BUILD_CACHE[n] = _build(n)
    nc = _BUILD_CACHE[n]

    dt = np.float32(1.0 / n) if n > 0 else np.float32(0.0)
    t0m = np.ascontiguousarray(dt * A.T, dtype=np.float32)
    d0 = _tiles_pk(np.ascontiguousarray(dt * A, dtype=np.float32))
    t0 = _tiles_pk(t0m)
    # Paterson-Stockmeyer blocks B_j = C(n,2j)*I + C(n,2j+1)*T0, j=1..4
    eye = np.eye(DZ, dtype=np.float32)
    c = [np.float32(comb(n, k)) for k in range(10)] if n > 0 else [np.float32(0)] * 10
    y4 = _tiles_pk((c[8] * eye + c[9] * t0m).astype(np.float32))
    bmats = [(c[2 * j] * eye + c[2 * j + 1] * t0m).astype(np.float32)
             for j in (1, 2, 3)]
    bsm = np.concatenate([_tiles_pk(m).reshape(P, DT, DZ) for m in bmats],
                         axis=1).reshape(P, 3 * DT * DZ)
    bsm = np.ascontiguousarray(bsm)
    w1t = _tiles_pk(np.ascontiguousarray(W1.T))           # [512, 2048]
    w2t = _tiles_pk(np.ascontiguousarray(W2.T))           # [2048, 10]
    b1t = np.ascontiguousarray(b1.reshape(HT, P).T)       # [128, 16]
    b2t = np.ascontiguousarray(b2.reshape(DY, 1))
    ident = np.eye(P, dtype=np.float32)

    in_maps = []
    for c in range(NCORES):
        xs = x[c * BC:(c + 1) * BC, :]                    # [512, 512]
        xt = _tiles_pk(np.ascontiguousarray(xs.T))        # [128, 4*512]
        in_maps.append({
            "xt": xt, "xtr": xt, "d0": d0, "t0": t0, "y4": y4, "bs": bsm,
            "w1t": w1t, "b1t": b1t,
            "w2t": w2t, "b2t": b2t, "ident": ident, "identr": ident,
        })

    trace = bool(os.environ.get("BASS_KERNEL_TRACE"))
    kwargs = {}
    if trace:
        kwargs = {"trace": True, "trace_cores": [0]}
    res = run_bass_kernel_spmd(nc, in_maps, list(range(NCORES)), **kwargs)
    if trace and res.exec_time_ns is not None:
        print(f"HW exec time: {res.exec_time_ns} ns")

    y = np.concatenate([res.results[c]["y"] for c in range(NCORES)], axis=0)
    return y.astype(np.float32)


# revision 38
# speedup vs baseline: 1.2607x; 1.1965x over previous
"""Trainium2 Bass kernel for MatrixOdeGradientDescentModel.

Reference computation (B=4096, DZ=512, H=2048, DY=10, n_steps=64):
    z = x; repeat n_steps: z += dt * z @ A.T          (dt = 1/n_steps)
    y = relu(z @ W1.T + b1) @ W2.T + b2

Algebraic rewrite: the Euler loop is linear, so
    z_final = x @ P^T with P^T = (W)^n,  W = I + dt*A^T  (T0 := dt*A^T).
(W)^n = sum_k C(n,k) T0^k. Since ||T0|| = ||A||/n (~0.014 here), the series
truncated at degree 9 has ~1e-7 relative tail for any n (C(n,k)/n^k <= 1/k!),
so we evaluate it Paterson-Stockmeyer style with X = T0^2:
    P_dev = c1*T0 + X*(B1 + X*(B2 + X*(B3 + X*B4)))     [P = I + P_dev]
where B_j = c_{2j}*I + c_{2j+1}*T0 are HOST-computed (O(n^2) elementwise) and
folded into the PSUM evictions, X is applied via its transpose D0^2
(D0 := dt*A, so D0^2 = (T0^2)^T is one matmul set away — no on-device
transposes). Then zT = xT + P_dev-apply(xT), and the MLP.

Sharding: data-parallel over batch. Each of the 8 cores gets 512 rows of x;
A/W1/W2 replicated; no cross-core communication.

Matmuls run in float32r (TF32-like, 4x faster than fp32 on the PE) with fp32
PSUM accumulation; the identity-free deviation formulation keeps the
end-to-end relative error at the ~2e-4 level.
"""

import os
from math import comb

import numpy as np

import concourse.bacc as bacc
import concourse.mybir as mybir
import concourse.tile as tile
from concourse.bass_utils import run_bass_kernel_spmd

P = 128
B, DZ, H, DY = 4096, 512, 2048, 10
NCORES = 8
BC = B // NCORES          # 512 rows per core
DT = DZ // P              # 4 k-tiles over DZ
HT = H // P               # 16 m-tiles over H

f32 = mybir.dt.float32
f32r = mybir.dt.float32r

_BUILD_CACHE = {}


def _emit_mm_set(nc, psum_pool, lhsT_tile, rhs_tile, evict, n_mt=DT):
    """One [512,512]-ish matmul set: for each output row-block mt, accumulate
    over DT k-tiles into PSUM and call evict(mt, psum_ap)."""
    for mt in range(n_mt):
        ps = psum_pool.tile([P, BC], f32, tag="ps")
        for kt in range(DT):
            nc.tensor.matmul(
                ps[:],
                lhsT_tile[:, kt, mt * P:(mt + 1) * P],
                rhs_tile[:, kt, :],
                start=(kt == 0),
                stop=(kt == DT - 1),
            )
        evict(mt, ps)


def _build(n_steps: int):
    """Build + compile the Bass module for a given n_steps."""
    n = int(n_steps)
    assert n >= 0
    nc = bacc.Bacc("TRN2", target_bir_lowering=False, debug=False,
                   enable_asserts=False, num_devices=NCORES)

    # f32r-declared DRAM inputs carry raw fp32 bytes; the PE rounds internally
    # (verified bit-identical to an explicit cast) so plain HWDGE DMA works.
    xt_d = nc.dram_tensor("xt", [P, DT * BC], f32, kind="ExternalInput")
    xtr_d = nc.dram_tensor("xtr", [P, DT * BC], f32r, kind="ExternalInput")
    t0_d = nc.dram_tensor("t0", [P, DT * DZ], f32r, kind="ExternalInput")
    w1t_d = nc.dram_tensor("w1t", [P, DT * H], f32r, kind="ExternalInput")
    b1t_d = nc.dram_tensor("b1t", [P, HT], f32, kind="ExternalInput")
    w2t_d = nc.dram_tensor("w2t", [P, HT * DY], f32r, kind="ExternalInput")
    b2t_d = nc.dram_tensor("b2t", [DY, 1], f32, kind="ExternalInput")
    ident_d = nc.dram_tensor("ident", [P, P], f32, kind="ExternalInput")
    identr_d = nc.dram_tensor("identr", [P, P], f32r, kind="ExternalInput")
    y_d = nc.dram_tensor("y", [BC, DY], f32, kind="ExternalOutput")

    mult = mybir.AluOpType.mult
    add = mybir.AluOpType.add
    c = [float(comb(n, k)) for k in range(10)]

    with tile.TileContext(nc) as tc:
        with (
            tc.tile_pool(name="const", bufs=1) as const_pool,
            tc.tile_pool(name="weights", bufs=1) as w_pool,
            tc.tile_pool(name="horner", bufs=2) as horner_pool,
            tc.tile_pool(name="bpool", bufs=2) as b_pool,
            tc.tile_pool(name="accp", bufs=2) as acc_pool,
            tc.tile_pool(name="acts", bufs=1) as act_pool,
            tc.tile_pool(name="out", bufs=2) as out_pool,
            tc.tile_pool(name="psum", bufs=7, space="PSUM") as psum_pool,
            tc.tile_pool(name="psum_y", bufs=1, space="PSUM") as psum_y_pool,
        ):
            # ---- loads: one HWDGE trigger queue, strict priority order -----
            # (DMA rings are FIFO and the two cores of an HBM stack share
            # ~350 GB/s, so chain-critical bytes must be enqueued first.)
            identr = const_pool.tile([P, P], f32r, tag="identr")
            nc.sync.dma_start(identr[:], identr_d.ap())
            t_cur = w_pool.tile([P, DT, DZ], f32r, tag="t0")
            t0_src = t0_d.ap().rearrange("p (t b) -> p t b", t=DT)
            for kt in range(DT):
                nc.sync.dma_start(t_cur[:, kt:kt + 1, :], t0_src[:, kt:kt + 1, :])

            def load(dram, shape, tag, dtype=f32r, chunks=1):
                r = w_pool.tile(shape, dtype, tag=tag)
                src = dram.ap().rearrange("p (t b) -> p t b", t=shape[1])
                for ch in range(chunks):
                    lo = shape[1] * ch // chunks
                    hi = shape[1] * (ch + 1) // chunks
                    nc.sync.dma_start(r[:, lo:hi, :], src[:, lo:hi, :])
                return r

            xt_r = load(xtr_d, [P, DT, BC], "xtr")
            xt = load(xt_d, [P, DT, BC], "xt", dtype=f32)
            w1t = load(w1t_d, [P, DT, H], "w1t", chunks=4)
            w2t = load(w2t_d, [P, HT, DY], "w2t")

            b1t = const_pool.tile([P, HT], f32, tag="b1t")
            nc.sync.dma_start(b1t[:], b1t_d.ap())
            b2t = const_pool.tile([DY, 1], f32, tag="b2t")
            nc.sync.dma_start(b2t[:], b2t_d.ap())
            ident = const_pool.tile([P, P], f32, tag="ident")
            nc.sync.dma_start(ident[:], ident_d.ap())

            # Brief PE warm-up while the t0 DMA streams: HAM only unthrottles
            # (1.2 -> 2.4 GHz) after ~3.4us of sustained matmul activity.
            ps_w0 = psum_y_pool.tile([P, P], f32, tag="psy")
            ps_w1 = psum_pool.tile([P, P], f32, tag="ps")
            for i in range(8):
                nc.tensor.matmul([ps_w0, ps_w1][i % 2][:], identr[:], identr[:],
                                 start=True, stop=True)

            # ---- D0 = T0^T via PE matmuls against the identity -------------
            # (saves a 1 MiB load on the DMA-critical front; also warms HAM)
            d_cur = w_pool.tile([P, DT, DZ], f32r, tag="d0")
            for a in range(DT):
                ps = psum_pool.tile([P, DZ], f32, tag="ps")
                for b in range(DT):
                    nc.tensor.matmul(
                        ps[:, b * P:(b + 1) * P],
                        t_cur[:, b, a * P:(a + 1) * P], identr[:],
                        start=True, stop=True)
                nc.scalar.activation(
                    d_cur[:, a, :], ps[:], mybir.ActivationFunctionType.Copy)

            # ---- scaled-diagonal helper (one reusable c*I big tile) --------
            cIbig = w_pool.tile([P, DT, DZ], f32, tag="cIbig")
            nc.gpsimd.memset(cIbig[:], 0.0)

            def set_diag(cv):
                for mt in range(DT):
                    nc.vector.tensor_scalar_mul(
                        cIbig[:, mt, mt * P:(mt + 1) * P], identr[:], cv)

            def make_b(cv_i, cv_t, dtype, tag):
                """B = cv_i * I + cv_t * T0, built on DVE off the PE path."""
                set_diag(cv_i)
                bt = b_pool.tile([P, DT, DZ], dtype, tag=tag)
                nc.vector.scalar_tensor_tensor(
                    bt[:], t_cur[:], cv_t, cIbig[:], op0=mult, op1=add)
                return bt

            acc = xt_r          # zT accumulator, fp32r [P, DT, BC]
            acc_f32 = xt        # exact fp32 twin for the fused +acc add

            def apply_T(t_tile, acc_r, acc_exact):
                """acc <- acc + P_dev-rows @ acc."""
                new_r = acc_pool.tile([P, DT, BC], f32r, tag="acc")

                def evict(mt, ps):
                    nc.vector.scalar_tensor_tensor(
                        new_r[:, mt, :], acc_exact[:, mt, :], 1.0, ps[:],
                        op0=mult, op1=add)

                _emit_mm_set(nc, psum_pool, t_tile, acc_r, evict)
                return new_r

            if n == 0:
                zt = xt_r
            elif n == 1:
                zt = apply_T(t_cur, acc, acc_f32)
            else:
                # ---- Paterson-Stockmeyer, X = T0^2, degree 9 --------------
                # Y4 first (needed earliest as the first Horner rhs).
                y4t = make_b(c[8], c[9], f32r, "y4")

                # X as its transpose D0^2 (the lhsT for X-products).
                x2 = w_pool.tile([P, DT, DZ], f32r, tag="x2")

                def evict_x2(mt, ps):
                    nc.scalar.activation(
                        x2[:, mt, :], ps[:], mybir.ActivationFunctionType.Copy)

                _emit_mm_set(nc, psum_pool, t_cur, d_cur, evict_x2)

                # Horner levels: Y_j = B_j + X @ Y_{j+1}.
                y_r = y4t
                for j in (3, 2, 1):
                    bj = make_b(c[2 * j], c[2 * j + 1], f32, "bj")
                    ynew = horner_pool.tile([P, DT, DZ], f32r, tag="ylev")

                    def evict_y(mt, ps, ynew=ynew, bj=bj):
                        nc.vector.scalar_tensor_tensor(
                            ynew[:, mt, :], bj[:, mt, :], 1.0, ps[:],
                            op0=mult, op1=add)

                    _emit_mm_set(nc, psum_pool, x2, y_r, evict_y)
                    y_r = ynew

                # P_dev = c1*T0 + X @ Y1  (c1 = n)
                pd = w_pool.tile([P, DT, DZ], f32r, tag="pd")

                def evict_pd(mt, ps):
                    nc.vector.scalar_tensor_tensor(
                        pd[:, mt, :], t_cur[:, mt, :], c[1], ps[:],
                        op0=mult, op1=add)

                _emit_mm_set(nc, psum_pool, x2, y_r, evict_pd)

                # zT = xT + P_dev-rows @ xT
                zt = apply_T(pd, acc, acc_f32)

            # ---- MLP: hT = relu(W1 @ z + b1); yT = W2 @ h + b2 -------------
            # Layer-2 accumulation MMs interleave with layer-1 so the tail
            # after the last h-tile is just one MM + bias + transpose.
            ht = act_pool.tile([P, HT, BC], f32r, tag="ht")
            ps_y = psum_y_pool.tile([DY, BC], f32, tag="psy")
            for mt in range(HT):
                ps = psum_pool.tile([P, BC], f32, tag="ps")
                for kt in range(DT):
                    nc.tensor.matmul(
                        ps[:], w1t[:, kt, mt * P:(mt + 1) * P], zt[:, kt, :],
                        start=(kt == 0), stop=(kt == DT - 1))
                nc.scalar.activation(
                    ht[:, mt, :], ps[:], mybir.ActivationFunctionType.Relu,
                    bias=b1t[:, mt:mt + 1])
                nc.tensor.matmul(ps_y[:], w2t[:, mt, :], ht[:, mt, :],
                                 start=(mt == 0), stop=(mt == HT - 1))
            ytb = out_pool.tile([DY, BC], f32, tag="ytb")
            nc.scalar.activation(ytb[:], ps_y[:],
                                 mybir.ActivationFunctionType.Identity,
                                 bias=b2t[:])

            # ---- transpose yT -> y and store -------------------------------
            y_sb = out_pool.tile([P, BC // P, DY], f32, tag="ysb")
            for bt in range(BC // P):
                ps_t = psum_y_pool.tile([P, DY], f32, tag="psy")
                nc.tensor.transpose(
                    ps_t[:], ytb[:, bt * P:(bt + 1) * P], ident[:DY, :DY])
                nc.vector.tensor_copy(y_sb[:, bt, :], ps_t[:])
            nc.sync.dma_start(
                y_d.ap().rearrange("(bt p) j -> p bt j", p=P), y_sb[:])

    nc.compile()
    return nc


def _tiles_pk(m: np.ndarray) -> np.ndarray:
    """[nt*128, C] -> [128, nt*C] partition-tiled layout (row r = kt*128+p)."""
    nt = m.shape[0] // P
    return np.ascontiguousarray(m.reshape(nt, P, -1).swapaxes(0, 1)).reshape(P, -1)


def kernel(x, A, W1, b1, W2, b2, n_steps) -> np.ndarray:
    x = np.asarray(x, dtype=np.float32)
    A = np.asarray(A, dtype=np.float32)
    W1 = np.asarray(W1, dtype=np.float32)
    b1 = np.asarray(b1, dtype=np.float32)
    W2 = np.asarray(W2, dtype=np.float32)
    b2 = np.asarray(b2, dtype=np.float32)
    n = int(np.asarray(n_steps))

    if n not in _BUILD_CACHE:
        _BUILD_CACHE[n] = _build(n)
    nc = _BUILD_CACHE[n]

    dt = np.float32(1.0 / n) if n > 0 else np.float32(0.0)
    t0 = _tiles_pk(np.ascontiguousarray(dt * A.T, dtype=np.float32))
    w1t = _tiles_pk(np.ascontiguousarray(W1.T))           # [512, 2048]
    w2t = _tiles_pk(np.ascontiguousarray(W2.T))           # [2048, 10]
    b1t = np.ascontiguousarray(b1.reshape(HT, P).T)       # [128, 16]
    b2t = np.ascontiguousarray(b2.reshape(DY, 1))
    ident = np.eye(P, dtype=np.float32)

    in_maps = []
    for c in range(NCORES):
        xs = x[c * BC:(c + 1) * BC, :]                    # [512, 512]
        xt = _tiles_pk(np.ascontiguousarray(xs.T))        # [128, 4*512]
        in_maps.append({
            "xt": xt, "xtr": xt, "t0": t0, "w1t": w1t, "b1t": b1t,
            "w2t": w2t, "b2t": b2t, "ident": ident, "identr": ident,
        })

    trace = bool(os.environ.get("BASS_KERNEL_TRACE"))
    kwargs = {}
    if trace:
        kwargs = {"trace": True, "trace_cores": [0]}
    res = run_bass_kernel_spmd(nc, in_maps, list(range(NCORES)), **kwargs)
    if trace and res.exec_time_ns is not None:
        print(f"HW exec time: {res.exec_time_ns} ns")

    y = np.concatenate([res.results[c]["y"] for c in range(NCORES)], axis=0)
    return y.astype(np.float32)
